# revision 1
# baseline (speedup 1.0000x reference)
"""Trainium2 Bass kernel for nn_CollaborativeExpertsWrapper.

Self-contained: shards batch B=128 across 8 NeuronCores (data-parallel
encoders), all-gathers [16, 2048] embeddings, each core redundantly computes
the masked ranking loss; host takes core 0's (loss, acc).
"""
import sys

sys.path.insert(0, "/opt/trn_rl_repo")

import math
from contextlib import ExitStack

import numpy as np

import concourse.bacc as bacc
import concourse.bass as bass
import concourse.mybir as mybir
import concourse.tile as tile
from concourse.alu_op_type import AluOpType
from concourse.masks import make_identity

F32 = mybir.dt.float32
F32R = mybir.dt.float32r
BF16 = mybir.dt.bfloat16
U8 = mybir.dt.uint8
AF = mybir.ActivationFunctionType
AX = mybir.AxisListType

N_CORES = 8
B = 128
BL = B // N_CORES  # 16 samples per core
T = 64
DIM = 512
HEADS = 4
HD = DIM // HEADS  # 128
MARGIN = 1.0
TOK = BL * T  # 1024 tokens per core per modality
O_T = 1024
ODIM = 512

_CACHE = {}


def _build():
    nc = bacc.Bacc("TRN2", target_bir_lowering=False, debug=False, num_devices=N_CORES)

    o_d = nc.dram_tensor("o", [BL, O_T, ODIM], F32, kind="ExternalInput").ap()
    rgb_d = nc.dram_tensor("rgb", [BL, T, 2048], F32, kind="ExternalInput").ap()
    aud_d = nc.dram_tensor("audio", [BL, T, 128], F32, kind="ExternalInput").ap()
    gm_d = nc.dram_tensor("group_mask", [B], U8, kind="ExternalInput").ap()

    wd = {}
    for m, dm in (("rgb", 2048), ("audio", 128)):
        for p in "qkv":
            wd[f"{m}_W{p}"] = nc.dram_tensor(f"{m}_W{p}", [dm, DIM], F32, kind="ExternalInput").ap()
            wd[f"{m}_b{p}"] = nc.dram_tensor(f"{m}_b{p}", [DIM], F32, kind="ExternalInput").ap()
        wd[f"{m}_Wo"] = nc.dram_tensor(f"{m}_Wo", [DIM, DIM], F32, kind="ExternalInput").ap()
        wd[f"{m}_bo"] = nc.dram_tensor(f"{m}_bo", [DIM], F32, kind="ExternalInput").ap()
        wd[f"{m}_W2"] = nc.dram_tensor(f"{m}_W2", [DIM, DIM], F32, kind="ExternalInput").ap()
        wd[f"{m}_b2"] = nc.dram_tensor(f"{m}_b2", [DIM], F32, kind="ExternalInput").ap()
    wd["expand_W"] = nc.dram_tensor("expand_W", [DIM, 2 * DIM], F32, kind="ExternalInput").ap()
    wd["expand_b"] = nc.dram_tensor("expand_b", [2 * DIM], F32, kind="ExternalInput").ap()

    out_d = nc.dram_tensor("out", [1, 2], F32, kind="ExternalOutput").ap()

    import os
    stage = os.environ.get("KSTAGE", "full")
    dbg_d = None
    if stage != "full":
        dbg_d = nc.dram_tensor("dbg", [B, 4 * DIM], F32, kind="ExternalOutput").ap()

    with tile.TileContext(nc) as tc:
        _emit(nc, tc, o_d, rgb_d, aud_d, gm_d, wd, out_d, stage, dbg_d)

    nc.compile()
    return nc


def _emit(nc, tc, o_d, rgb_d, aud_d, gm_d, wd, out_d, stage="full", dbg_d=None):
    stk = ExitStack()
    with stk:
        const = stk.enter_context(tc.tile_pool(name="const", bufs=1))
        persist = stk.enter_context(tc.tile_pool(name="persist", bufs=1))
        ps = stk.enter_context(tc.tile_pool(name="psum", bufs=7, space="PSUM"))
        dram = stk.enter_context(tc.tile_pool(name="dram", bufs=1, space="DRAM"))

        def pst(shape, tag="ps", bufs=None):
            return ps.tile(shape, F32, tag=tag, bufs=bufs, name=tag)

        # ---------------- constants ----------------
        ident = const.tile([128, 128], F32, tag="ident")
        make_identity(nc, ident)
        ones_col_f32 = const.tile([128, 1], F32, tag="ones_col_f32")
        nc.vector.memset(ones_col_f32[:], 1.0)
        ones64_s = const.tile([128, 128], F32, tag="ones64_s")
        nc.vector.memset(ones64_s[:], 0.0)
        nc.vector.memset(ones64_s[0:64, 0:64], 1.0)
        nc.vector.memset(ones64_s[64:128, 64:128], 1.0)
        ones64_r = const.tile([128, 128], F32R, tag="ones64")
        nc.vector.tensor_copy(ones64_r[:], ones64_s[:])
        ones_row_f32 = const.tile([1, 128], F32, tag="ones_row_f32")
        nc.vector.memset(ones_row_f32[:], 1.0)
        ones128 = const.tile([128, 128], F32, tag="ones128")
        nc.vector.memset(ones128[:], 1.0)
        ones_row_r = const.tile([1, 128], F32R, tag="ones_row_r")
        nc.vector.tensor_copy(ones_row_r[:], ones_row_f32[:])
        sel16_s = const.tile([128, BL, BL], F32, tag="sel16_s")
        nc.vector.memset(sel16_s[:], 0.0)
        for b in range(BL):
            nc.vector.memset(sel16_s[:, b, b : b + 1], 1.0)
        sel16 = const.tile([128, BL, BL], BF16, tag="sel16")
        nc.vector.tensor_copy(sel16[:], sel16_s[:])
        ones64_bf = const.tile([64, 64], BF16, tag="ones64_bf")
        nc.vector.tensor_copy(ones64_bf[:], ones64_s[0:64, 0:64])

        g_row_u8 = const.tile([1, B], U8, tag="g_row_u8")
        nc.sync.dma_start(g_row_u8[:], gm_d[None, :])
        g_row = const.tile([1, B], F32, tag="g_row")
        nc.vector.tensor_copy(g_row[:], g_row_u8[:])
        g_col_u8 = const.tile([B, 1], U8, tag="g_col_u8")
        nc.sync.dma_start(g_col_u8[:], gm_d[:, None])
        g_col = const.tile([B, 1], F32, tag="g_col")
        nc.vector.tensor_copy(g_col[:], g_col_u8[:])
        gneg_row = const.tile([1, B], F32, tag="gneg_row")
        nc.vector.tensor_scalar(gneg_row[:], g_row[:], 1e30, -1e30, AluOpType.mult, AluOpType.add)

        feat_sb = persist.tile([BL, 2 * DIM], F32, tag="feat")
        oo_sb = persist.tile([BL, 2 * DIM], F32, tag="oo")

        # o tiles pool opened early so its space never WAR-blocks on encoder pools
        o_pool = stk.enter_context(tc.tile_pool(name="o_pool", bufs=3))

        # ---------------- rgb encoder (its weight DMAs queue ahead of the o stream) ------
        _encoder(nc, tc, pst, persist, const, "rgb", 2048, rgb_d, wd, feat_sb, 0,
                 ident, ones_row_r, ones64_bf)

        # expand weights loaded early (small; unblocks the expand chain)
        expw_pool = stk.enter_context(tc.tile_pool(name="expw", bufs=1))
        expw = expw_pool.tile([128, 4, 2 * DIM], F32R, tag="expw")
        nc.gpsimd.dma_start(expw[:], wd["expand_W"].rearrange("(c p) d -> p c d", p=128))
        expb = expw_pool.tile([1, 2 * DIM], F32R, tag="expb")
        nc.gpsimd.dma_start(expb[:], wd["expand_b"][None, :])

        # ---------------- audio encoder ----------------
        _encoder(nc, tc, pst, persist, const, "audio", 128, aud_d, wd,
                 feat_sb, DIM, ident, ones_row_r, ones64_bf)

        if stage == "enc":
            nc.sync.dma_start(dbg_d[0:BL, 0 : 2 * DIM], feat_sb[:])
            return

        # ---------------- o-mean (bf16 stream, overlaps encoder tail) ----------------
        om_ps = pst([BL, ODIM], tag="ps_om", bufs=1)
        o_view = o_d.rearrange("b (n p) d -> b p n d", p=128)
        for b in range(BL):
            o_sb = o_pool.tile([128, O_T // 128, ODIM], BF16, tag="o_tile")
            nc.gpsimd.dma_start(o_sb[:], o_view[b])
            for j in range(O_T // 128):
                nc.tensor.matmul(
                    om_ps[:],
                    sel16[:, b, :],
                    o_sb[:, j, :],
                    start=(b == 0 and j == 0),
                    stop=(b == BL - 1 and j == O_T // 128 - 1),
                )

        om_sb = persist.tile([BL, ODIM], F32, tag="om")
        nc.scalar.activation(om_sb[:], om_ps[:], AF.Copy, scale=1.0 / O_T)
        omT = persist.tile([128, 4, BL], F32R, tag="omT")
        for c in range(4):
            tp = pst([128, BL])
            nc.tensor.transpose(tp[:], om_sb[:, 128 * c : 128 * (c + 1)], ident[:BL, :BL])
            nc.scalar.copy(omT[:, c, :], tp[:])

        # ---------------- expand + normalize -> oo ----------------
        if True:
            oo_ps = []
            for half in range(2):
                pp = pst([BL, DIM])
                for c in range(4):
                    nc.tensor.matmul(pp[:], omT[:, c, :], expw[:, c, 512 * half : 512 * (half + 1)],
                                     start=(c == 0), stop=False)
                nc.tensor.matmul(pp[:], ones_row_r[:, :BL], expb[:, 512 * half : 512 * (half + 1)],
                                 start=False, stop=True)
                oo_ps.append(pp)
            sq_junk = persist.tile([BL, DIM], F32, tag="sq_junk")
            ss = [persist.tile([BL, 1], F32, tag=f"ss{i}", name=f"ss{i}") for i in range(2)]
            for half in range(2):
                nc.scalar.activation(sq_junk[:], oo_ps[half][:], AF.Square, accum_out=ss[half][:])
            nrm = persist.tile([BL, 1], F32, tag="nrm")
            nc.vector.tensor_tensor(nrm[:], ss[0][:], ss[1][:], AluOpType.add)
            nc.scalar.sqrt(nrm[:], nrm[:])
            nc.vector.tensor_scalar_max(nrm[:], nrm[:], 1e-12)
            rnrm = persist.tile([BL, 1], F32, tag="rnrm")
            nc.vector.reciprocal(rnrm[:], nrm[:])
            for half in range(2):
                nc.vector.tensor_scalar_mul(oo_sb[:, 512 * half : 512 * (half + 1)],
                                            oo_ps[half][:], rnrm[:])


        if stage == "oenc":
            nc.sync.dma_start(dbg_d[0:BL, 0 : 2 * DIM], feat_sb[:])
            nc.sync.dma_start(dbg_d[0:BL, 2 * DIM :], oo_sb[:])
            return

        # ---------------- AllGather ----------------
        ag_in = dram.tile([BL, 4 * DIM], F32)
        ag_out = dram.tile([B, 4 * DIM], F32)
        nc.sync.dma_start(ag_in[:, : 2 * DIM], feat_sb[:])
        nc.sync.dma_start(ag_in[:, 2 * DIM :], oo_sb[:])
        import os
        if os.environ.get("KTIME"):
            # collective-free stand-in for TimelineSim (cost model can't model
            # collectives); timing-equivalent except the ~15us AllGather.
            nc.sync.dma_start(ag_out[0:BL, :], ag_in[:])
        else:
            nc.gpsimd.collective_compute(
                "AllGather",
                AluOpType.bypass,
                replica_groups=[list(range(N_CORES))],
                ins=[ag_in.opt()],
                outs=[ag_out.opt()],
            )

        # ---------------- ranking ----------------
        with tc.tile_pool(name="rank", bufs=1) as rank_pool:
            emb = rank_pool.tile([B, 4 * DIM], F32, tag="emb")
            nc.sync.dma_start(emb[:], ag_out[:])

            if stage == "ag":
                nc.sync.dma_start(dbg_d[:], emb[:])
                return

            # transpose emb -> embT [128, 16, 128]; chunks 0..7 featT, 8..15 ooT
            embT = rank_pool.tile([128, 16, 128], F32, tag="embT")
            for grp4 in range(4):
                tp = pst([128, 512])
                for j in range(4):
                    c = 4 * grp4 + j
                    nc.tensor.transpose(tp[:, 128 * j : 128 * (j + 1)],
                                        emb[:, 128 * c : 128 * (c + 1)], ident[:])
                nc.scalar.copy(embT[:, 4 * grp4 : 4 * grp4 + 4, :],
                               tp[:].rearrange("p (j c) -> p j c", j=4))

            G_ps = pst([B, B])
            for c in range(8):
                nc.tensor.matmul(G_ps[:], embT[:, 8 + c, :], embT[:, c, :],
                                 start=(c == 0), stop=(c == 7))
            G_sb = rank_pool.tile([B, B], F32, tag="G_sb")
            nc.scalar.copy(G_sb[:], G_ps[:])

            if stage == "rank1":
                nc.sync.dma_start(dbg_d[:, 0:B], G_sb[:])
                return

            junk = rank_pool.tile([B, B], F32, tag="junk")
            diag = rank_pool.tile([B, 1], F32, tag="diag")
            nc.vector.tensor_tensor(junk[:], G_sb[:], ident[:], AluOpType.mult)
            nc.vector.reduce_sum(diag[:], junk[:], axis=AX.X)
            mdiag = rank_pool.tile([B, 1], F32, tag="mdiag")
            nc.vector.tensor_scalar(mdiag[:], diag[:], -1.0, MARGIN,
                                    AluOpType.mult, AluOpType.add)

            Gt_ps = pst([B, B])
            nc.tensor.transpose(Gt_ps[:], G_sb[:], ident[:])
            Gt_sb = rank_pool.tile([B, B], F32, tag="Gt_sb")
            nc.scalar.copy(Gt_sb[:], Gt_ps[:])

            if stage == "rank1b":
                nc.sync.dma_start(dbg_d[:, 0:B], Gt_sb[:])
                nc.sync.dma_start(dbg_d[:, B : B + 1], diag[:])
                return

            # broadcast g along partitions: gb[m, n] = g[n], via colsums of a
            # zero-padded one-row matrix (K=1 matmuls are avoided).
            g_pad = rank_pool.tile([B, B], F32, tag="g_pad")
            nc.vector.memset(g_pad[:], 0.0)
            nc.vector.tensor_copy(g_pad[0:1, :], g_row[:])
            gb_ps = pst([B, B])
            nc.tensor.matmul(gb_ps[:], ones128[:], g_pad[:], start=True, stop=True)
            gneg_sb = rank_pool.tile([B, B], F32, tag="gneg_sb")
            nc.vector.tensor_scalar(gneg_sb[:], gb_ps[:], 1e30, -1e30,
                                    AluOpType.mult, AluOpType.add)

            stack = rank_pool.tile([B, 6], F32, tag="stack")
            Gm = rank_pool.tile([B, B], F32, tag="Gm")
            rmax = rank_pool.tile([B, 1], F32, tag="rmax")
            top = rank_pool.tile([B, 1], F32, tag="top")
            w = rank_pool.tile([B, 1], F32, tag="w")
            sel = rank_pool.tile([B, 1], F32, tag="sel")
            eq = rank_pool.tile([B, 1], F32, tag="eq")
            colv = rank_pool.tile([B, 1], F32, tag="colv")

            for di, Gsrc in enumerate((G_sb, Gt_sb)):
                T_sb = rank_pool.tile([B, B], F32, tag=f"T{di}")
                nc.scalar.activation(T_sb[:], Gsrc[:], AF.Relu, bias=mdiag[:])
                nc.vector.tensor_tensor(junk[:], T_sb[:], gb_ps[:], AluOpType.mult)
                nc.vector.reduce_sum(w[:], junk[:], axis=AX.X)
                nc.vector.tensor_tensor(stack[:, di : di + 1], w[:], g_col[:], AluOpType.mult)
                nc.vector.tensor_tensor(Gm[:], Gsrc[:], gneg_sb[:], AluOpType.add)
                nc.vector.reduce_max(rmax[:], Gm[:], axis=AX.X)
                nc.vector.tensor_tensor(top[:], diag[:], rmax[:], AluOpType.is_ge)
                nc.vector.tensor_tensor(junk[:], Gsrc[:], gb_ps[:], AluOpType.mult)
                nc.vector.reduce_sum(sel[:], junk[:], axis=AX.X)
                nc.vector.tensor_tensor(sel[:], sel[:], g_col[:], AluOpType.mult)
                nc.vector.tensor_scalar(eq[:], sel[:], 0.0, None, AluOpType.is_equal)
                nc.vector.tensor_scalar(colv[:], eq[:], -1.0, 1.0,
                                        AluOpType.mult, AluOpType.add)
                nc.vector.tensor_copy(stack[:, 4 + di : 5 + di], colv[:])
                nc.vector.tensor_tensor(stack[:, 2 + di : 3 + di], colv[:], top[:],
                                        AluOpType.mult)

            if stage == "rank2":
                nc.sync.dma_start(dbg_d[:, 0:6], stack[:])
                nc.sync.dma_start(dbg_d[:, 8:136], Gt_sb[:])
                return

            S_ps = pst([1, 6])
            nc.tensor.matmul(S_ps[:], ones_col_f32[:], stack[:], start=True, stop=True)
            S_sb = rank_pool.tile([1, 6], F32, tag="S_sb")
            nc.vector.tensor_copy(S_sb[:], S_ps[:])

            if stage == "rank3":
                nc.sync.dma_start(dbg_d[0:1, 0:6], S_sb[:])
                return

            sg = rank_pool.tile([1, 1], F32, tag="sg")
            nc.vector.reduce_sum(sg[:], g_row[:], axis=AX.X)

            def sc(tag):
                return rank_pool.tile([1, 1], F32, tag=tag, name=tag)

            t_ls = sc("t_ls")
            nc.vector.tensor_tensor(t_ls[:], S_sb[:, 0:1], S_sb[:, 1:2], AluOpType.add)
            num = sc("num")
            nc.vector.tensor_scalar_mul(num[:], sg[:], -2.0 * MARGIN)
            nc.vector.tensor_tensor(num[:], num[:], t_ls[:], AluOpType.add)
            d1 = sc("d1")
            nc.vector.tensor_scalar(d1[:], sg[:], -1.0, 1.0, AluOpType.add, AluOpType.max)
            ind = sc("ind")
            nc.vector.tensor_scalar(ind[:], sg[:], -1.0, 0.0, AluOpType.add, AluOpType.max)
            nc.vector.tensor_scalar_min(ind[:], ind[:], 1.0)
            nv = sc("nv")
            nc.vector.tensor_tensor(nv[:], ind[:], sg[:], AluOpType.mult)
            d2 = sc("d2")
            nc.vector.tensor_scalar_max(d2[:], nv[:], 1.0)
            r1 = sc("r1")
            nc.vector.reciprocal(r1[:], d1[:])
            r2 = sc("r2")
            nc.vector.reciprocal(r2[:], d2[:])
            out_sb = rank_pool.tile([1, 2], F32, tag="out_sb")
            nc.vector.tensor_tensor(num[:], num[:], r1[:], AluOpType.mult)
            nc.vector.tensor_tensor(out_sb[:, 0:1], num[:], r2[:], AluOpType.mult)

            acc_h = []
            for di in range(2):
                nvx = sc(f"nvx{di}")
                nc.vector.tensor_scalar_max(nvx[:], S_sb[:, 4 + di : 5 + di], 1.0)
                rx = sc(f"rx{di}")
                nc.vector.reciprocal(rx[:], nvx[:])
                ax = sc(f"ax{di}")
                nc.vector.tensor_tensor(ax[:], S_sb[:, 2 + di : 3 + di], rx[:], AluOpType.mult)
                acc_h.append(ax)
            asum = sc("asum")
            nc.vector.tensor_tensor(asum[:], acc_h[0][:], acc_h[1][:], AluOpType.add)
            nc.vector.tensor_scalar_mul(out_sb[:, 1:2], asum[:], 0.5)

            nc.sync.dma_start(out_d[:], out_sb[:])


def _encoder(nc, tc, pst, persist, const, mod, dm, x_d, wd, feat_sb, feat_off,
             ident, ones_row_r, ones64_bf):
    """Self-attention pooled encoder; writes feat_sb[:, feat_off:feat_off+512]."""
    K = dm // 128
    n_tt = TOK // 128  # 8

    enc_pool_cm = tc.tile_pool(name=f"enc_{mod}", bufs=1)
    enc = enc_pool_cm.__enter__()
    qT = enc.tile([128, HEADS, TOK], BF16, tag="qT")
    kT = enc.tile([128, HEADS, TOK], BF16, tag="kT")
    v_sb = enc.tile([128, n_tt, DIM], BF16, tag="v_sb")
    poolT = enc.tile([128, HEADS, BL], F32R, tag="poolT")

    with ExitStack() as estk:
        xT_pool = estk.enter_context(tc.tile_pool(name=f"xT_{mod}", bufs=1))
        xT = xT_pool.tile([128, K, TOK], F32R, tag="xT")
        flat = x_d.rearrange("b t d -> (b t) d")
        with tc.tile_pool(name=f"xload_{mod}", bufs=2) as xload:
            if dm == 128:
                x_nat = xload.tile([128, n_tt, 128], F32, tag="x_nat_a")
                nc.sync.dma_start(x_nat[:], flat.rearrange("(n p) d -> p n d", p=128))
                for tt in range(n_tt):
                    tp = pst([128, 512])
                    nc.tensor.transpose(tp[:, :128], x_nat[:, tt, :], ident[:])
                    nc.scalar.copy(xT[:, 0, 128 * tt : 128 * (tt + 1)], tp[:, :128])
            else:
                for tt in range(n_tt):
                    x_nat = xload.tile([128, dm], F32, tag="x_nat")
                    nc.sync.dma_start(x_nat[:], flat[128 * tt : 128 * (tt + 1), :])
                    for kc4 in range(K // 4):
                        tp = pst([128, 512])
                        for j in range(4):
                            kc = 4 * kc4 + j
                            nc.tensor.transpose(tp[:, 128 * j : 128 * (j + 1)],
                                                x_nat[:, 128 * kc : 128 * (kc + 1)], ident[:])
                        nc.scalar.copy(xT[:, 4 * kc4 : 4 * kc4 + 4, 128 * tt : 128 * (tt + 1)],
                                       tp[:].rearrange("p (j c) -> p j c", j=4))

        # v: lhsT = xT token-tile (stationary), rhs = Wv k-rows (moving)
        with tc.tile_pool(name=f"wv_{mod}", bufs=1) as wv_pool:
            wv = wv_pool.tile([128, K, DIM], F32R, tag="wv")
            nc.gpsimd.dma_start(wv[:], wd[f"{mod}_Wv"].rearrange("(kc p) d -> p kc d", p=128))
            bv = wv_pool.tile([1, DIM], F32R, tag="bv")
            nc.gpsimd.dma_start(bv[:], wd[f"{mod}_bv"][None, :])
            for tt in range(n_tt):
                pv = pst([128, DIM])
                for kc in range(K):
                    nc.tensor.matmul(pv[:], xT[:, kc, 128 * tt : 128 * (tt + 1)], wv[:, kc, :],
                                     start=(kc == 0), stop=False)
                nc.tensor.matmul(pv[:], ones_row_r[:], bv[:], start=False, stop=True)
                nc.vector.tensor_copy(v_sb[:, tt, :], pv[:])

        # q, k: lhsT = W column-block (stationary), rhs = xT (moving) -> [d, tok]
        bq_sb = const.tile([128, HEADS], F32, tag=f"bq_{mod}")
        nc.sync.dma_start(bq_sb[:], wd[f"{mod}_bq"].rearrange("(o p) -> p o", p=128))
        bk_sb = const.tile([128, HEADS], F32, tag=f"bk_{mod}")
        nc.sync.dma_start(bk_sb[:], wd[f"{mod}_bk"].rearrange("(o p) -> p o", p=128))
        with tc.tile_pool(name=f"wcol_{mod}", bufs=2) as wcol_pool:
            for pname, outT, b_sb in (("q", qT, bq_sb), ("k", kT, bk_sb)):
                w_d = wd[f"{mod}_W{pname}"].rearrange("(kc p) d -> p kc d", p=128)
                if K == 1:
                    wfull = wcol_pool.tile([128, DIM], F32R, tag="wfull", name="wfull")
                    nc.gpsimd.dma_start(wfull[:], w_d[:, 0, :])
                for dt_ in range(HEADS):
                    if K == 1:
                        wcol = wfull[:, None, 128 * dt_ : 128 * (dt_ + 1)]
                    else:
                        wcol = wcol_pool.tile([128, K, 128], F32R, tag="wcol",
                                              name="wcol")
                        nc.gpsimd.dma_start(wcol[:],
                                            w_d[:, :, 128 * dt_ : 128 * (dt_ + 1)])
                    for blk in range(TOK // 512):
                        pq = pst([128, 512])
                        for kc in range(K):
                            nc.tensor.matmul(pq[:], wcol[:, kc, :],
                                             xT[:, kc, 512 * blk : 512 * (blk + 1)],
                                             start=(kc == 0), stop=(kc == K - 1))
                        nc.scalar.activation(outT[:, dt_, 512 * blk : 512 * (blk + 1)], pq[:],
                                             AF.Identity, bias=b_sb[:, dt_ : dt_ + 1])

    # attention, grp-outer: reshuffle 8 samples of v to base partition 0 via
    # SBUF->SBUF DMA (engines cannot shift partitions), then per-head flow.
    scale = 1.0 / math.sqrt(HD)
    with ExitStack() as lstk:
        late = lstk.enter_context(tc.tile_pool(name=f"late_{mod}", bufs=1))
        avT = late.tile([128, HEADS, TOK], F32R, tag="avT")
        wo_pool = lstk.enter_context(tc.tile_pool(name=f"wo_{mod}", bufs=1))
        wo = wo_pool.tile([128, HEADS, DIM], F32R, tag="wo")
        nc.gpsimd.dma_start(wo[:], wd[f"{mod}_Wo"].rearrange("(h p) d -> p h d", p=128))
        w2 = wo_pool.tile([128, HEADS, DIM], F32R, tag="w2")
        nc.gpsimd.dma_start(w2[:], wd[f"{mod}_W2"].rearrange("(c p) d -> p c d", p=128))
        b2 = wo_pool.tile([1, DIM], F32R, tag="b2")
        nc.gpsimd.dma_start(b2[:], wd[f"{mod}_b2"][None, :])
        bo_sb = const.tile([128, HEADS], F32, tag=f"bo_{mod}")
        nc.sync.dma_start(bo_sb[:], wd[f"{mod}_bo"].rearrange("(o p) -> p o", p=128))
        ap = lstk.enter_context(tc.tile_pool(name=f"attn_{mod}", bufs=3))
        for grp in range(BL // 8):
            v8 = ap.tile([64, 8, DIM], BF16, tag="v8")
            v8v = v8[:].rearrange("p (ul half) d -> p ul half d", half=2)
            nc.sync.dma_start(v8v[:, :, 0, :], v_sb[0:64, 4 * grp : 4 * grp + 4, :])
            nc.sync.dma_start(v8v[:, :, 1, :], v_sb[64:128, 4 * grp : 4 * grp + 4, :])
            for h in range(HEADS):
                sT8 = pst([64, 512])
                for i in range(8):
                    b = 8 * grp + i
                    nc.tensor.matmul(sT8[:, 64 * i : 64 * (i + 1)],
                                     kT[:, h, 64 * b : 64 * (b + 1)],
                                     qT[:, h, 64 * b : 64 * (b + 1)],
                                     start=True, stop=True)
                exps = ap.tile([64, 512], BF16, tag="exps")
                nc.scalar.activation(exps[:], sT8[:], AF.Exp, scale=scale)
                rs = pst([64, 512])
                nc.tensor.matmul(rs[:], ones64_bf[:], exps[:],
                                 start=True, stop=True)
                rrs = ap.tile([64, 512], F32, tag="rrs")
                nc.vector.reciprocal(rrs[:], rs[:])
                aT8 = ap.tile([64, 512], BF16, tag="aT8")
                nc.vector.tensor_tensor(aT8[:], exps[:], rrs[:], AluOpType.mult)
                avp = pst([128, 512])
                for i in range(8):
                    nc.tensor.matmul(avp[:, 64 * i : 64 * (i + 1)],
                                     v8[:, i, 128 * h : 128 * (h + 1)],
                                     aT8[:, 64 * i : 64 * (i + 1)],
                                     start=True, stop=True)
                nc.vector.tensor_copy(avT[:, h, 512 * grp : 512 * (grp + 1)], avp[:])

        # out-proj (transposed) + time pooling + W2
        _proj_w2(nc, tc, pst, wo_pool, mod, feat_sb, feat_off, avT, poolT,
                 ones_row_r, wo, w2, b2, bo_sb)

    enc_pool_cm.__exit__(None, None, None)


def _proj_w2(nc, tc, pst, wo_pool, mod, feat_sb, feat_off, avT, poolT,
             ones_row_r, wo, w2, b2, bo_sb):
    if True:
        red = wo_pool.tile([128, 8], F32, tag="red")
        for dt_ in range(HEADS):
            for blk in range(TOK // 512):
                pp = pst([128, 512])
                for h in range(HEADS):
                    nc.tensor.matmul(pp[:], wo[:, h, 128 * dt_ : 128 * (dt_ + 1)],
                                     avT[:, h, 512 * blk : 512 * (blk + 1)],
                                     start=(h == 0), stop=(h == HEADS - 1))
                nc.vector.reduce_sum(red[:], pp[:].rearrange("p (s t) -> p s t", t=T),
                                     axis=AX.X)
                nc.vector.tensor_scalar(poolT[:, dt_, 8 * blk : 8 * blk + 8], red[:],
                                        1.0 / T, bo_sb[:, dt_ : dt_ + 1],
                                        AluOpType.mult, AluOpType.add)

        pf = pst([BL, DIM])
        for c in range(HEADS):
            nc.tensor.matmul(pf[:], poolT[:, c, :], w2[:, c, :], start=(c == 0), stop=False)
        nc.tensor.matmul(pf[:], ones_row_r[:, :BL], b2[:], start=False, stop=True)
        nc.scalar.copy(feat_sb[:, feat_off : feat_off + DIM], pf[:])


def kernel(**inputs):
    if "runner" not in _CACHE:
        _CACHE["runner"] = _make_runner()
    return _CACHE["runner"](inputs)


def _make_runner():
    nc = _build()
    import jax
    from jax.sharding import Mesh, PartitionSpec
    from jax.experimental.shard_map import shard_map
    from concourse import bass2jax

    bass2jax.install_neuronx_cc_hook()

    partition_name = nc.partition_id_tensor.name if nc.partition_id_tensor else None
    in_names, out_names, out_avals, zero_outs = [], [], [], []
    for alloc in nc.m.functions[0].allocations:
        if not isinstance(alloc, mybir.MemoryLocationSet):
            continue
        name = alloc.memorylocations[0].name
        if alloc.kind == "ExternalInput":
            if name != partition_name:
                in_names.append(name)
        elif alloc.kind == "ExternalOutput":
            out_names.append(name)
            shape = tuple(alloc.tensor_shape)
            dtype = mybir.dt.np(alloc.dtype)
            out_avals.append(jax.core.ShapedArray(shape, dtype))
            zero_outs.append(np.zeros(shape, dtype))
    n_params = len(in_names)
    all_in_names = list(in_names) + list(out_names)
    if partition_name is not None:
        all_in_names.append(partition_name)

    def _body(*args):
        operands = list(args)
        if partition_name is not None:
            operands.append(bass2jax.partition_id_tensor())
        outs = bass2jax._bass_exec_p.bind(
            *operands,
            out_avals=tuple(out_avals),
            in_names=tuple(all_in_names),
            out_names=tuple(out_names),
            lowering_input_output_aliases=(),
            sim_require_finite=True,
            sim_require_nnan=True,
            nc=nc,
        )
        return tuple(outs)

    devices = jax.devices()[:N_CORES]
    mesh = Mesh(np.asarray(devices), ("core",))
    in_specs = (PartitionSpec("core"),) * (n_params + len(out_names))
    out_specs = (PartitionSpec("core"),) * len(out_names)
    sharded = jax.jit(
        shard_map(_body, mesh=mesh, in_specs=in_specs, out_specs=out_specs,
                  check_rep=False),
        keep_unused=True,
    )

    out_idx = out_names.index("out")

    def run(inputs):
        per_core = _shard_inputs(inputs)
        concat_in = [
            np.concatenate([per_core[c][name] for c in range(N_CORES)], axis=0)
            for name in in_names
        ]
        concat_zeros = [
            np.zeros((N_CORES * z.shape[0], *z.shape[1:]), z.dtype) for z in zero_outs
        ]
        out_arrs = sharded(*concat_in, *concat_zeros)
        run.last_outputs = {n: np.asarray(out_arrs[i]) for i, n in enumerate(out_names)}
        out = run.last_outputs["out"]  # [8, 2]
        return np.float32(out[0, 0]), np.float32(out[0, 1])

    run.sharded = sharded
    run.in_names = in_names
    run.zero_outs = zero_outs
    run.nc = nc
    return run


def _shard_inputs(inputs):
    per_core = []
    gm = np.ascontiguousarray(np.asarray(inputs["group_mask"]).astype(np.uint8))
    shared = {}
    for k, v in inputs.items():
        if k not in ("o", "rgb", "audio", "group_mask"):
            shared[k] = np.ascontiguousarray(np.asarray(v, dtype=np.float32))
    o = np.asarray(inputs["o"], dtype=np.float32)
    rgb = np.asarray(inputs["rgb"], dtype=np.float32)
    audio = np.asarray(inputs["audio"], dtype=np.float32)
    for c in range(N_CORES):
        sl = slice(BL * c, BL * (c + 1))
        m = {
            "o": np.ascontiguousarray(o[sl]),
            "rgb": np.ascontiguousarray(rgb[sl]),
            "audio": np.ascontiguousarray(audio[sl]),
            "group_mask": gm,
        }
        m.update(shared)
        per_core.append(m)
    return per_core



# revision 12
# speedup vs baseline: 1.1373x; 1.1373x over previous
"""Trainium2 Bass kernel for nn_CollaborativeExpertsWrapper.

Self-contained: shards batch B=128 across 8 NeuronCores (data-parallel
encoders), all-gathers [16, 2048] embeddings, each core redundantly computes
the masked ranking loss; host takes core 0's (loss, acc).

v2: full-bf16 datapath — inputs and weights are cast to bf16 on the host
(halves HBM traffic), xT is produced by HWDGE transpose-DMA (removes the
PE-transpose + ACT-copy pipeline), all projection matmuls run in bf16.
Accumulation stays fp32 in PSUM; the ranking block stays fp32.
"""
import sys

sys.path.insert(0, "/opt/trn_rl_repo")

import math
from contextlib import ExitStack

import numpy as np

import concourse.bacc as bacc
import concourse.bass as bass
import concourse.mybir as mybir
import concourse.tile as tile
from concourse.alu_op_type import AluOpType
from concourse.masks import make_identity

F32 = mybir.dt.float32
F32R = mybir.dt.float32r
BF16 = mybir.dt.bfloat16
U8 = mybir.dt.uint8
AF = mybir.ActivationFunctionType
AX = mybir.AxisListType

N_CORES = 8
B = 128
BL = B // N_CORES  # 16 samples per core
T = 64
DIM = 512
HEADS = 4
HD = DIM // HEADS  # 128
MARGIN = 1.0
TOK = BL * T  # 1024 tokens per core per modality
O_T = 1024
ODIM = 512

_CACHE = {}


def _build():
    nc = bacc.Bacc("TRN2", target_bir_lowering=False, debug=False, num_devices=N_CORES)

    o_d = nc.dram_tensor("o", [BL, O_T, ODIM], BF16, kind="ExternalInput").ap()
    rgb_d = nc.dram_tensor("rgb", [BL, T, 2048], BF16, kind="ExternalInput").ap()
    aud_d = nc.dram_tensor("audio", [BL, T, 128], BF16, kind="ExternalInput").ap()
    gm_d = nc.dram_tensor("group_mask", [B], U8, kind="ExternalInput").ap()

    wd = {}
    for m, dm in (("rgb", 2048), ("audio", 128)):
        for p in "qkv":
            wd[f"{m}_W{p}"] = nc.dram_tensor(f"{m}_W{p}", [dm, DIM], BF16, kind="ExternalInput").ap()
            wd[f"{m}_b{p}"] = nc.dram_tensor(f"{m}_b{p}", [DIM], F32, kind="ExternalInput").ap()
        wd[f"{m}_Wo"] = nc.dram_tensor(f"{m}_Wo", [DIM, DIM], BF16, kind="ExternalInput").ap()
        wd[f"{m}_bo"] = nc.dram_tensor(f"{m}_bo", [DIM], F32, kind="ExternalInput").ap()
        wd[f"{m}_W2"] = nc.dram_tensor(f"{m}_W2", [DIM, DIM], BF16, kind="ExternalInput").ap()
        wd[f"{m}_b2"] = nc.dram_tensor(f"{m}_b2", [DIM], F32, kind="ExternalInput").ap()
    wd["expand_W"] = nc.dram_tensor("expand_W", [DIM, 2 * DIM], BF16, kind="ExternalInput").ap()
    wd["expand_b"] = nc.dram_tensor("expand_b", [2 * DIM], F32, kind="ExternalInput").ap()

    out_d = nc.dram_tensor("out", [1, 2], F32, kind="ExternalOutput").ap()

    import os
    stage = os.environ.get("KSTAGE", "full")
    dbg_d = None
    if stage != "full":
        dbg_d = nc.dram_tensor("dbg", [B, 4 * DIM], F32, kind="ExternalOutput").ap()

    with tile.TileContext(nc) as tc:
        _emit(nc, tc, o_d, rgb_d, aud_d, gm_d, wd, out_d, stage, dbg_d)

    nc.compile()
    return nc


def _emit(nc, tc, o_d, rgb_d, aud_d, gm_d, wd, out_d, stage="full", dbg_d=None):
    stk = ExitStack()
    with stk:
        const = stk.enter_context(tc.tile_pool(name="const", bufs=1))
        persist = stk.enter_context(tc.tile_pool(name="persist", bufs=1))
        ps = stk.enter_context(tc.tile_pool(name="psum", bufs=7, space="PSUM"))
        dram = stk.enter_context(tc.tile_pool(name="dram", bufs=1, space="DRAM"))

        def pst(shape, tag="ps", bufs=None):
            return ps.tile(shape, F32, tag=tag, bufs=bufs, name=tag)

        # ---------------- constants ----------------
        ident = const.tile([128, 128], F32, tag="ident")
        make_identity(nc, ident)
        ones_col_f32 = const.tile([128, 1], F32, tag="ones_col_f32")
        nc.vector.memset(ones_col_f32[:], 1.0)
        ones64_s = const.tile([128, 128], F32, tag="ones64_s")
        nc.vector.memset(ones64_s[:], 0.0)
        nc.vector.memset(ones64_s[0:64, 0:64], 1.0)
        nc.vector.memset(ones64_s[64:128, 64:128], 1.0)
        ones_row_f32 = const.tile([1, 128], F32, tag="ones_row_f32")
        nc.vector.memset(ones_row_f32[:], 1.0)
        ones128 = const.tile([128, 128], F32, tag="ones128")
        nc.vector.memset(ones128[:], 1.0)
        ones_row_bf = const.tile([1, 128], BF16, tag="ones_row_bf")
        nc.vector.tensor_copy(ones_row_bf[:], ones_row_f32[:])
        sel16_s = const.tile([128, BL, BL], F32, tag="sel16_s")
        nc.vector.memset(sel16_s[:], 0.0)
        for b in range(BL):
            nc.vector.memset(sel16_s[:, b, b : b + 1], 1.0)
        sel16 = const.tile([128, BL, BL], BF16, tag="sel16")
        nc.vector.tensor_copy(sel16[:], sel16_s[:])
        ones64_bf = const.tile([64, 64], BF16, tag="ones64_bf")
        nc.vector.tensor_copy(ones64_bf[:], ones64_s[0:64, 0:64])

        g_row_u8 = const.tile([1, B], U8, tag="g_row_u8")
        nc.sync.dma_start(g_row_u8[:], gm_d[None, :])
        g_row = const.tile([1, B], F32, tag="g_row")
        nc.vector.tensor_copy(g_row[:], g_row_u8[:])
        g_col_u8 = const.tile([B, 1], U8, tag="g_col_u8")
        nc.sync.dma_start(g_col_u8[:], gm_d[:, None])
        g_col = const.tile([B, 1], F32, tag="g_col")
        nc.vector.tensor_copy(g_col[:], g_col_u8[:])

        feat_sb = persist.tile([BL, 2 * DIM], F32, tag="feat")
        oo_sb = persist.tile([BL, 2 * DIM], F32, tag="oo")

        # o tiles pool opened early so its space never WAR-blocks on encoder pools
        o_pool = stk.enter_context(tc.tile_pool(name="o_pool", bufs=4))

        # ---------------- rgb encoder (its weight DMAs queue ahead of the o stream) ------
        rgb_enc = _encoder_qkv(nc, tc, pst, persist, const, "rgb", 2048, rgb_d, wd,
                               ident, ones_row_bf)
        _attention(nc, tc, pst, const, "rgb", wd, feat_sb, 0, ones_row_bf,
                   ones64_bf, rgb_enc)

        # expand weights loaded early (small; unblocks the expand chain)
        expw_pool = stk.enter_context(tc.tile_pool(name="expw", bufs=1))
        expw = expw_pool.tile([128, 4, 2 * DIM], BF16, tag="expw")
        nc.gpsimd.dma_start(expw[:], wd["expand_W"].rearrange("(c p) d -> p c d", p=128))
        expb = expw_pool.tile([1, 2 * DIM], BF16, tag="expb")
        nc.gpsimd.dma_start(expb[:], wd["expand_b"][None, :])

        # ---------------- audio encoder qkv (attention deferred past o-mean) ---------
        aud_enc = _encoder_qkv(nc, tc, pst, persist, const, "audio", 128, aud_d, wd,
                               ident, ones_row_bf)

        # ---------------- o-mean (bf16 stream; 2-sample DMAs split Pool/SP) ----------
        om_ps = pst([BL, ODIM], tag="ps_om", bufs=1)
        o_view = o_d.rearrange("b (n p) d -> p b n d", p=128)  # [128, 16, 8, 512]
        PAIR = 2
        for c in range(BL // PAIR):
            o_sb = o_pool.tile([128, PAIR, O_T // 128, ODIM], BF16, tag="o_tile")
            eng = nc.gpsimd if c % 2 == 0 else nc.sync
            eng.dma_start(o_sb[:], o_view[:, PAIR * c : PAIR * (c + 1)])
            for bb in range(PAIR):
                b = PAIR * c + bb
                for j in range(O_T // 128):
                    nc.tensor.matmul(
                        om_ps[:],
                        sel16[:, b, :],
                        o_sb[:, bb, j, :],
                        start=(b == 0 and j == 0),
                        stop=(b == BL - 1 and j == O_T // 128 - 1),
                    )

        om_sb = persist.tile([BL, ODIM], F32, tag="om")
        nc.scalar.activation(om_sb[:], om_ps[:], AF.Copy, scale=1.0 / O_T)
        omT = persist.tile([128, 4, BL], BF16, tag="omT")
        for c in range(4):
            tp = pst([128, BL])
            nc.tensor.transpose(tp[:], om_sb[:, 128 * c : 128 * (c + 1)], ident[:BL, :BL])
            nc.scalar.copy(omT[:, c, :], tp[:])

        # ---------------- expand + normalize -> oo ----------------
        if True:
            oo_ps = []
            for half in range(2):
                pp = pst([BL, DIM])
                for c in range(4):
                    nc.tensor.matmul(pp[:], omT[:, c, :], expw[:, c, 512 * half : 512 * (half + 1)],
                                     start=(c == 0), stop=False)
                nc.tensor.matmul(pp[:], ones_row_bf[:, :BL], expb[:, 512 * half : 512 * (half + 1)],
                                 start=False, stop=True)
                oo_ps.append(pp)
            sq_junk = persist.tile([BL, DIM], F32, tag="sq_junk")
            ss = [persist.tile([BL, 1], F32, tag=f"ss{i}", name=f"ss{i}") for i in range(2)]
            for half in range(2):
                nc.scalar.activation(sq_junk[:], oo_ps[half][:], AF.Square, accum_out=ss[half][:])
            nrm = persist.tile([BL, 1], F32, tag="nrm")
            nc.vector.tensor_tensor(nrm[:], ss[0][:], ss[1][:], AluOpType.add)
            nc.scalar.sqrt(nrm[:], nrm[:])
            nc.vector.tensor_scalar_max(nrm[:], nrm[:], 1e-12)
            rnrm = persist.tile([BL, 1], F32, tag="rnrm")
            nc.vector.reciprocal(rnrm[:], nrm[:])
            for half in range(2):
                nc.vector.tensor_scalar_mul(oo_sb[:, 512 * half : 512 * (half + 1)],
                                            oo_ps[half][:], rnrm[:])

        # ---------------- audio attention (overlaps o-mean/expand tail) -------------
        _attention(nc, tc, pst, const, "audio", wd, feat_sb, DIM, ones_row_bf,
                   ones64_bf, aud_enc)

        if stage == "enc":
            nc.sync.dma_start(dbg_d[0:BL, 0 : 2 * DIM], feat_sb[:])
            return

        if stage == "oenc":
            nc.sync.dma_start(dbg_d[0:BL, 0 : 2 * DIM], feat_sb[:])
            nc.sync.dma_start(dbg_d[0:BL, 2 * DIM :], oo_sb[:])
            return

        # ---------------- AllGather ----------------
        ag_in = dram.tile([BL, 4 * DIM], F32)
        ag_out = dram.tile([B, 4 * DIM], F32)
        nc.sync.dma_start(ag_in[:, : 2 * DIM], feat_sb[:])
        nc.sync.dma_start(ag_in[:, 2 * DIM :], oo_sb[:])
        import os
        if os.environ.get("KTIME"):
            # collective-free stand-in for TimelineSim (cost model can't model
            # collectives); timing-equivalent except the ~15us AllGather.
            nc.sync.dma_start(ag_out[0:BL, :], ag_in[:])
        else:
            nc.gpsimd.collective_compute(
                "AllGather",
                AluOpType.bypass,
                replica_groups=[list(range(N_CORES))],
                ins=[ag_in.opt()],
                outs=[ag_out.opt()],
            )

        # ---------------- ranking ----------------
        with tc.tile_pool(name="rank", bufs=1) as rank_pool:
            emb = rank_pool.tile([B, 4 * DIM], F32, tag="emb")
            nc.sync.dma_start(emb[:], ag_out[:])

            if stage == "ag":
                nc.sync.dma_start(dbg_d[:], emb[:])
                return

            # transpose emb -> embT [128, 16, 128]; chunks 0..7 featT, 8..15 ooT
            embT = rank_pool.tile([128, 16, 128], F32, tag="embT")
            for grp4 in range(4):
                tp = pst([128, 512])
                for j in range(4):
                    c = 4 * grp4 + j
                    nc.tensor.transpose(tp[:, 128 * j : 128 * (j + 1)],
                                        emb[:, 128 * c : 128 * (c + 1)], ident[:])
                nc.scalar.copy(embT[:, 4 * grp4 : 4 * grp4 + 4, :],
                               tp[:].rearrange("p (j c) -> p j c", j=4))

            G_ps = pst([B, B])
            for c in range(8):
                nc.tensor.matmul(G_ps[:], embT[:, 8 + c, :], embT[:, c, :],
                                 start=(c == 0), stop=(c == 7))
            G_sb = rank_pool.tile([B, B], F32, tag="G_sb")
            nc.scalar.copy(G_sb[:], G_ps[:])

            if stage == "rank1":
                nc.sync.dma_start(dbg_d[:, 0:B], G_sb[:])
                return

            junk = rank_pool.tile([B, B], F32, tag="junk")
            diag = rank_pool.tile([B, 1], F32, tag="diag")
            nc.vector.tensor_tensor(junk[:], G_sb[:], ident[:], AluOpType.mult)
            nc.vector.reduce_sum(diag[:], junk[:], axis=AX.X)
            mdiag = rank_pool.tile([B, 1], F32, tag="mdiag")
            nc.vector.tensor_scalar(mdiag[:], diag[:], -1.0, MARGIN,
                                    AluOpType.mult, AluOpType.add)

            Gt_ps = pst([B, B])
            nc.tensor.transpose(Gt_ps[:], G_sb[:], ident[:])
            Gt_sb = rank_pool.tile([B, B], F32, tag="Gt_sb")
            nc.scalar.copy(Gt_sb[:], Gt_ps[:])

            if stage == "rank1b":
                nc.sync.dma_start(dbg_d[:, 0:B], Gt_sb[:])
                nc.sync.dma_start(dbg_d[:, B : B + 1], diag[:])
                return

            # broadcast g along partitions: gb[m, n] = g[n], via colsums of a
            # zero-padded one-row matrix (K=1 matmuls are avoided).
            g_pad = rank_pool.tile([B, B], F32, tag="g_pad")
            nc.vector.memset(g_pad[:], 0.0)
            nc.vector.tensor_copy(g_pad[0:1, :], g_row[:])
            gb_ps = pst([B, B])
            nc.tensor.matmul(gb_ps[:], ones128[:], g_pad[:], start=True, stop=True)
            gneg_sb = rank_pool.tile([B, B], F32, tag="gneg_sb")
            nc.vector.tensor_scalar(gneg_sb[:], gb_ps[:], 1e30, -1e30,
                                    AluOpType.mult, AluOpType.add)

            stack = rank_pool.tile([B, 6], F32, tag="stack")
            Gm = rank_pool.tile([B, B], F32, tag="Gm")
            rmax = rank_pool.tile([B, 1], F32, tag="rmax")
            top = rank_pool.tile([B, 1], F32, tag="top")
            w = rank_pool.tile([B, 1], F32, tag="w")
            sel = rank_pool.tile([B, 1], F32, tag="sel")
            eq = rank_pool.tile([B, 1], F32, tag="eq")
            colv = rank_pool.tile([B, 1], F32, tag="colv")

            for di, (Gsrc, GsrcT) in enumerate(((G_sb, Gt_sb), (Gt_sb, G_sb))):
                T_sb = rank_pool.tile([B, B], F32, tag=f"T{di}")
                nc.scalar.activation(T_sb[:], Gsrc[:], AF.Relu, bias=mdiag[:])
                nc.vector.tensor_tensor(junk[:], T_sb[:], gb_ps[:], AluOpType.mult)
                nc.vector.reduce_sum(w[:], junk[:], axis=AX.X)
                nc.vector.tensor_tensor(stack[:, di : di + 1], w[:], g_col[:], AluOpType.mult)
                nc.vector.tensor_tensor(Gm[:], Gsrc[:], gneg_sb[:], AluOpType.add)
                nc.vector.reduce_max(rmax[:], Gm[:], axis=AX.X)
                nc.vector.tensor_tensor(top[:], diag[:], rmax[:], AluOpType.is_ge)
                # sel[i] = sum_b Gsrc[i,b]*g[b] as an N=1 matmul off GsrcT
                sel_ps = pst([B, 1], tag="ps_sel")
                nc.tensor.matmul(sel_ps[:], GsrcT[:], g_col[:], start=True, stop=True)
                nc.vector.tensor_tensor(sel[:], sel_ps[:], g_col[:], AluOpType.mult)
                nc.vector.tensor_scalar(eq[:], sel[:], 0.0, None, AluOpType.is_equal)
                nc.vector.tensor_scalar(colv[:], eq[:], -1.0, 1.0,
                                        AluOpType.mult, AluOpType.add)
                nc.vector.tensor_copy(stack[:, 4 + di : 5 + di], colv[:])
                nc.vector.tensor_tensor(stack[:, 2 + di : 3 + di], colv[:], top[:],
                                        AluOpType.mult)

            if stage == "rank2":
                nc.sync.dma_start(dbg_d[:, 0:6], stack[:])
                nc.sync.dma_start(dbg_d[:, 8:136], Gt_sb[:])
                return

            S_ps = pst([1, 6])
            nc.tensor.matmul(S_ps[:], ones_col_f32[:], stack[:], start=True, stop=True)
            S_sb = rank_pool.tile([1, 6], F32, tag="S_sb")
            nc.vector.tensor_copy(S_sb[:], S_ps[:])

            if stage == "rank3":
                nc.sync.dma_start(dbg_d[0:1, 0:6], S_sb[:])
                return

            sg = rank_pool.tile([1, 1], F32, tag="sg")
            nc.vector.reduce_sum(sg[:], g_row[:], axis=AX.X)

            def sc(tag):
                return rank_pool.tile([1, 1], F32, tag=tag, name=tag)

            t_ls = sc("t_ls")
            nc.vector.tensor_tensor(t_ls[:], S_sb[:, 0:1], S_sb[:, 1:2], AluOpType.add)
            num = sc("num")
            nc.vector.tensor_scalar_mul(num[:], sg[:], -2.0 * MARGIN)
            nc.vector.tensor_tensor(num[:], num[:], t_ls[:], AluOpType.add)
            d1 = sc("d1")
            nc.vector.tensor_scalar(d1[:], sg[:], -1.0, 1.0, AluOpType.add, AluOpType.max)
            ind = sc("ind")
            nc.vector.tensor_scalar(ind[:], sg[:], -1.0, 0.0, AluOpType.add, AluOpType.max)
            nc.vector.tensor_scalar_min(ind[:], ind[:], 1.0)
            nv = sc("nv")
            nc.vector.tensor_tensor(nv[:], ind[:], sg[:], AluOpType.mult)
            d2 = sc("d2")
            nc.vector.tensor_scalar_max(d2[:], nv[:], 1.0)
            r1 = sc("r1")
            nc.vector.reciprocal(r1[:], d1[:])
            r2 = sc("r2")
            nc.vector.reciprocal(r2[:], d2[:])
            out_sb = rank_pool.tile([1, 2], F32, tag="out_sb")
            nc.vector.tensor_tensor(num[:], num[:], r1[:], AluOpType.mult)
            nc.vector.tensor_tensor(out_sb[:, 0:1], num[:], r2[:], AluOpType.mult)

            acc_h = []
            for di in range(2):
                nvx = sc(f"nvx{di}")
                nc.vector.tensor_scalar_max(nvx[:], S_sb[:, 4 + di : 5 + di], 1.0)
                rx = sc(f"rx{di}")
                nc.vector.reciprocal(rx[:], nvx[:])
                ax = sc(f"ax{di}")
                nc.vector.tensor_tensor(ax[:], S_sb[:, 2 + di : 3 + di], rx[:], AluOpType.mult)
                acc_h.append(ax)
            asum = sc("asum")
            nc.vector.tensor_tensor(asum[:], acc_h[0][:], acc_h[1][:], AluOpType.add)
            nc.vector.tensor_scalar_mul(out_sb[:, 1:2], asum[:], 0.5)

            nc.sync.dma_start(out_d[:], out_sb[:])


def _encoder_qkv(nc, tc, pst, persist, const, mod, dm, x_d, wd,
                 ident, ones_row_bf):
    """Self-attention encoder, projection part: computes qT/kT/v_sb."""
    K = dm // 128
    n_tt = TOK // 128  # 8

    enc_pool_cm = tc.tile_pool(name=f"enc_{mod}", bufs=1)
    enc = enc_pool_cm.__enter__()
    qT = enc.tile([128, HEADS, TOK], BF16, tag="qT")
    kT = enc.tile([128, HEADS, TOK], BF16, tag="kT")
    v_sb = enc.tile([128, n_tt, DIM], BF16, tag="v_sb")
    poolT = enc.tile([128, HEADS, BL], BF16, tag="poolT")

    with ExitStack() as estk:
        xT_pool = estk.enter_context(tc.tile_pool(name=f"xT_{mod}", bufs=1))
        xT = xT_pool.tile([128, K, TOK], BF16, tag="xT")
        flat = x_d.rearrange("b t d -> (b t) d")
        # xT[d, tok] straight from DRAM via HWDGE xbar-transpose (bf16).
        # One transpose-DMA per 128-token row-block so downstream matmuls can
        # start after the first block lands; alternate SP/ACT queues.
        if K > 1:
            for tt in range(n_tt):
                eng = nc.sync if tt % 2 == 0 else nc.scalar
                eng.dma_start(xT[:, :, 128 * tt : 128 * (tt + 1)],
                              flat[128 * tt : 128 * (tt + 1), :], transpose=True)
        else:
            nc.sync.dma_start(xT[:, 0, :], flat[:, :], transpose=True)

        # v: lhsT = xT token-tile (stationary), rhs = Wv k-rows (moving)
        with tc.tile_pool(name=f"wv_{mod}", bufs=1) as wv_pool:
            wv = wv_pool.tile([128, K, DIM], BF16, tag="wv")
            wv_view = wd[f"{mod}_Wv"].rearrange("(kc p) d -> p kc d", p=128)
            # chunked so the first v matmuls unblock after ~1/4 of the load
            kchunk = max(K // 4, 1)
            for c0 in range(0, K, kchunk):
                nc.gpsimd.dma_start(wv[:, c0 : c0 + kchunk, :],
                                    wv_view[:, c0 : c0 + kchunk, :])
            bv = wv_pool.tile([1, DIM], BF16, tag="bv")
            nc.gpsimd.dma_start(bv[:], wd[f"{mod}_bv"][None, :])
            for tt in range(n_tt):
                pv = pst([128, DIM])
                for kc in range(K):
                    nc.tensor.matmul(pv[:], xT[:, kc, 128 * tt : 128 * (tt + 1)], wv[:, kc, :],
                                     start=(kc == 0), stop=False)
                nc.tensor.matmul(pv[:], ones_row_bf[:], bv[:], start=False, stop=True)
                nc.vector.tensor_copy(v_sb[:, tt, :], pv[:])

        # q, k: lhsT = W column-block (stationary), rhs = xT (moving) -> [d, tok]
        bq_sb = const.tile([128, HEADS], F32, tag=f"bq_{mod}")
        nc.sync.dma_start(bq_sb[:], wd[f"{mod}_bq"].rearrange("(o p) -> p o", p=128))
        bk_sb = const.tile([128, HEADS], F32, tag=f"bk_{mod}")
        nc.sync.dma_start(bk_sb[:], wd[f"{mod}_bk"].rearrange("(o p) -> p o", p=128))
        with tc.tile_pool(name=f"wcol_{mod}", bufs=2) as wcol_pool:
            for pname, outT, b_sb in (("q", qT, bq_sb), ("k", kT, bk_sb)):
                w_d = wd[f"{mod}_W{pname}"].rearrange("(kc p) d -> p kc d", p=128)
                if K == 1:
                    wfull = wcol_pool.tile([128, DIM], BF16, tag="wfull", name="wfull")
                    nc.gpsimd.dma_start(wfull[:], w_d[:, 0, :])
                for dt_ in range(HEADS):
                    if K == 1:
                        wcol = wfull[:, None, 128 * dt_ : 128 * (dt_ + 1)]
                    else:
                        wcol = wcol_pool.tile([128, K, 128], BF16, tag="wcol",
                                              name="wcol")
                        nc.gpsimd.dma_start(wcol[:],
                                            w_d[:, :, 128 * dt_ : 128 * (dt_ + 1)])
                    for blk in range(TOK // 512):
                        pq = pst([128, 512])
                        for kc in range(K):
                            nc.tensor.matmul(pq[:], wcol[:, kc, :],
                                             xT[:, kc, 512 * blk : 512 * (blk + 1)],
                                             start=(kc == 0), stop=(kc == K - 1))
                        nc.scalar.activation(outT[:, dt_, 512 * blk : 512 * (blk + 1)], pq[:],
                                             AF.Identity, bias=b_sb[:, dt_ : dt_ + 1])

    return {"cm": enc_pool_cm, "qT": qT, "kT": kT, "v_sb": v_sb, "poolT": poolT}


def _attention(nc, tc, pst, const, mod, wd, feat_sb, feat_off, ones_row_bf,
               ones64_bf, enc):
    """Attention + pooling + out-proj; software-pipelined emission so the
    per-(grp,head) exp->sum->recip->mult chain overlaps across iterations.
    Writes feat_sb[:, feat_off:feat_off+512]."""
    qT, kT, v_sb, poolT = enc["qT"], enc["kT"], enc["v_sb"], enc["poolT"]
    scale = 1.0 / math.sqrt(HD)
    with ExitStack() as lstk:
        late = lstk.enter_context(tc.tile_pool(name=f"late_{mod}", bufs=1))
        avT = late.tile([128, HEADS, TOK], BF16, tag="avT")
        wo_pool = lstk.enter_context(tc.tile_pool(name=f"wo_{mod}", bufs=1))
        wo = wo_pool.tile([128, HEADS, DIM], BF16, tag="wo")
        nc.gpsimd.dma_start(wo[:], wd[f"{mod}_Wo"].rearrange("(h p) d -> p h d", p=128))
        w2 = wo_pool.tile([128, HEADS, DIM], BF16, tag="w2")
        nc.gpsimd.dma_start(w2[:], wd[f"{mod}_W2"].rearrange("(c p) d -> p c d", p=128))
        b2 = wo_pool.tile([1, DIM], BF16, tag="b2")
        nc.gpsimd.dma_start(b2[:], wd[f"{mod}_b2"][None, :])
        bo_sb = const.tile([128, HEADS], F32, tag=f"bo_{mod}")
        nc.sync.dma_start(bo_sb[:], wd[f"{mod}_bo"].rearrange("(o p) -> p o", p=128))
        ap = lstk.enter_context(tc.tile_pool(name=f"attn_{mod}", bufs=3))

        NIT = (BL // 8) * HEADS
        v8s, exps_t, sT8_t, rs_t, aT8_t = {}, {}, {}, {}, {}

        def stage_a(t):  # scores + exp
            grp, h = divmod(t, HEADS)
            if h == 0:
                v8 = ap.tile([64, 8, DIM], BF16, tag="v8")
                v8v = v8[:].rearrange("p (ul half) d -> p ul half d", half=2)
                nc.sync.dma_start(v8v[:, :, 0, :], v_sb[0:64, 4 * grp : 4 * grp + 4, :])
                nc.sync.dma_start(v8v[:, :, 1, :], v_sb[64:128, 4 * grp : 4 * grp + 4, :])
                v8s[grp] = v8
            sT8 = pst([64, 512])
            for i in range(8):
                b = 8 * grp + i
                nc.tensor.matmul(sT8[:, 64 * i : 64 * (i + 1)],
                                 kT[:, h, 64 * b : 64 * (b + 1)],
                                 qT[:, h, 64 * b : 64 * (b + 1)],
                                 start=True, stop=True)
            exps = ap.tile([64, 512], BF16, tag="exps")
            nc.scalar.activation(exps[:], sT8[:], AF.Exp, scale=scale)
            exps_t[t] = exps

        def stage_b(t):  # rowsum + reciprocal + normalize
            rs = pst([64, 512])
            nc.tensor.matmul(rs[:], ones64_bf[:], exps_t[t][:], start=True, stop=True)
            rrs = ap.tile([64, 512], F32, tag="rrs")
            nc.vector.reciprocal(rrs[:], rs[:])
            aT8 = ap.tile([64, 512], BF16, tag="aT8")
            nc.vector.tensor_tensor(aT8[:], exps_t[t][:], rrs[:], AluOpType.mult)
            aT8_t[t] = aT8

        def stage_c(t):  # attention-weighted v + copy out
            grp, h = divmod(t, HEADS)
            v8 = v8s[grp]
            avp = pst([128, 512])
            for i in range(8):
                nc.tensor.matmul(avp[:, 64 * i : 64 * (i + 1)],
                                 v8[:, i, 128 * h : 128 * (h + 1)],
                                 aT8_t[t][:, 64 * i : 64 * (i + 1)],
                                 start=True, stop=True)
            nc.vector.tensor_copy(avT[:, h, 512 * grp : 512 * (grp + 1)], avp[:])

        for t in range(NIT + 2):
            if t < NIT:
                stage_a(t)
            if 0 <= t - 1 < NIT:
                stage_b(t - 1)
            if 0 <= t - 2 < NIT:
                stage_c(t - 2)

        # out-proj (transposed) + time pooling + W2
        _proj_w2(nc, tc, pst, wo_pool, mod, feat_sb, feat_off, avT, poolT,
                 ones_row_bf, wo, w2, b2, bo_sb)

    enc["cm"].__exit__(None, None, None)


def _proj_w2(nc, tc, pst, wo_pool, mod, feat_sb, feat_off, avT, poolT,
             ones_row_bf, wo, w2, b2, bo_sb):
    if True:
        red = wo_pool.tile([128, 8], F32, tag="red")
        for dt_ in range(HEADS):
            for blk in range(TOK // 512):
                pp = pst([128, 512])
                for h in range(HEADS):
                    nc.tensor.matmul(pp[:], wo[:, h, 128 * dt_ : 128 * (dt_ + 1)],
                                     avT[:, h, 512 * blk : 512 * (blk + 1)],
                                     start=(h == 0), stop=(h == HEADS - 1))
                nc.vector.reduce_sum(red[:], pp[:].rearrange("p (s t) -> p s t", t=T),
                                     axis=AX.X)
                nc.vector.tensor_scalar(poolT[:, dt_, 8 * blk : 8 * blk + 8], red[:],
                                        1.0 / T, bo_sb[:, dt_ : dt_ + 1],
                                        AluOpType.mult, AluOpType.add)

        pf = pst([BL, DIM])
        for c in range(HEADS):
            nc.tensor.matmul(pf[:], poolT[:, c, :], w2[:, c, :], start=(c == 0), stop=False)
        nc.tensor.matmul(pf[:], ones_row_bf[:, :BL], b2[:], start=False, stop=True)
        nc.scalar.copy(feat_sb[:, feat_off : feat_off + DIM], pf[:])


def kernel(**inputs):
    if "runner" not in _CACHE:
        _CACHE["runner"] = _make_runner()
    return _CACHE["runner"](inputs)


def _make_runner():
    nc = _build()
    import jax
    from jax.sharding import Mesh, PartitionSpec
    from jax.experimental.shard_map import shard_map
    from concourse import bass2jax

    bass2jax.install_neuronx_cc_hook()

    partition_name = nc.partition_id_tensor.name if nc.partition_id_tensor else None
    in_names, out_names, out_avals, zero_outs = [], [], [], []
    in_dtypes = {}
    for alloc in nc.m.functions[0].allocations:
        if not isinstance(alloc, mybir.MemoryLocationSet):
            continue
        name = alloc.memorylocations[0].name
        if alloc.kind == "ExternalInput":
            if name != partition_name:
                in_names.append(name)
                in_dtypes[name] = mybir.dt.np(alloc.dtype)
        elif alloc.kind == "ExternalOutput":
            out_names.append(name)
            shape = tuple(alloc.tensor_shape)
            dtype = mybir.dt.np(alloc.dtype)
            out_avals.append(jax.core.ShapedArray(shape, dtype))
            zero_outs.append(np.zeros(shape, dtype))
    n_params = len(in_names)
    all_in_names = list(in_names) + list(out_names)
    if partition_name is not None:
        all_in_names.append(partition_name)

    def _body(*args):
        operands = list(args)
        if partition_name is not None:
            operands.append(bass2jax.partition_id_tensor())
        outs = bass2jax._bass_exec_p.bind(
            *operands,
            out_avals=tuple(out_avals),
            in_names=tuple(all_in_names),
            out_names=tuple(out_names),
            lowering_input_output_aliases=(),
            sim_require_finite=True,
            sim_require_nnan=True,
            nc=nc,
        )
        return tuple(outs)

    devices = jax.devices()[:N_CORES]
    mesh = Mesh(np.asarray(devices), ("core",))
    in_specs = (PartitionSpec("core"),) * (n_params + len(out_names))
    out_specs = (PartitionSpec("core"),) * len(out_names)
    sharded = jax.jit(
        shard_map(_body, mesh=mesh, in_specs=in_specs, out_specs=out_specs,
                  check_rep=False),
        keep_unused=True,
    )

    out_idx = out_names.index("out")

    def run(inputs):
        per_core = _shard_inputs(inputs, in_dtypes)
        concat_in = [
            np.concatenate([per_core[c][name] for c in range(N_CORES)], axis=0)
            for name in in_names
        ]
        concat_zeros = [
            np.zeros((N_CORES * z.shape[0], *z.shape[1:]), z.dtype) for z in zero_outs
        ]
        out_arrs = sharded(*concat_in, *concat_zeros)
        run.last_outputs = {n: np.asarray(out_arrs[i]) for i, n in enumerate(out_names)}
        out = run.last_outputs["out"]  # [8, 2]
        return np.float32(out[0, 0]), np.float32(out[0, 1])

    run.sharded = sharded
    run.in_names = in_names
    run.in_dtypes = in_dtypes
    run.zero_outs = zero_outs
    run.nc = nc
    return run


def _shard_inputs(inputs, in_dtypes=None):
    if in_dtypes is None:
        in_dtypes = _CACHE["runner"].in_dtypes
    per_core = []
    gm = np.ascontiguousarray(np.asarray(inputs["group_mask"]).astype(np.uint8))
    shared = {}
    for k, v in inputs.items():
        if k not in ("o", "rgb", "audio", "group_mask"):
            shared[k] = np.ascontiguousarray(
                np.asarray(v).astype(in_dtypes.get(k, np.float32)))
    o = np.asarray(inputs["o"]).astype(in_dtypes.get("o", np.float32))
    rgb = np.asarray(inputs["rgb"]).astype(in_dtypes.get("rgb", np.float32))
    audio = np.asarray(inputs["audio"]).astype(in_dtypes.get("audio", np.float32))
    for c in range(N_CORES):
        sl = slice(BL * c, BL * (c + 1))
        m = {
            "o": np.ascontiguousarray(o[sl]),
            "rgb": np.ascontiguousarray(rgb[sl]),
            "audio": np.ascontiguousarray(audio[sl]),
            "group_mask": gm,
        }
        m.update(shared)
        per_core.append(m)
    return per_core


# revision 22
# speedup vs baseline: 19.1952x; 16.8783x over previous
"""Trainium2 Bass kernel for nn_CollaborativeExpertsWrapper.

Self-contained: shards batch B=128 across 8 NeuronCores (data-parallel
encoders), all-gathers [16, 2048] embeddings, each core redundantly computes
the masked ranking loss; host takes core 0's (loss, acc).

v2: full-bf16 datapath — inputs and weights are cast to bf16 on the host
(halves HBM traffic), xT is produced by HWDGE transpose-DMA (removes the
PE-transpose + ACT-copy pipeline), all projection matmuls run in bf16.
Accumulation stays fp32 in PSUM; the ranking block stays fp32.
"""
import sys

sys.path.insert(0, "/opt/trn_rl_repo")

import math
from contextlib import ExitStack

import numpy as np

import concourse.bacc as bacc
import concourse.bass as bass
import concourse.mybir as mybir
import concourse.tile as tile
from concourse.alu_op_type import AluOpType
from concourse.masks import make_identity

F32 = mybir.dt.float32
F32R = mybir.dt.float32r
BF16 = mybir.dt.bfloat16
U8 = mybir.dt.uint8
AF = mybir.ActivationFunctionType
AX = mybir.AxisListType

N_CORES = 8
B = 128
BL = B // N_CORES  # 16 samples per core
T = 64
DIM = 512
HEADS = 4
HD = DIM // HEADS  # 128
MARGIN = 1.0
TOK = BL * T  # 1024 tokens per core per modality
O_T = 1024
ODIM = 512

_CACHE = {}


def _build():
    nc = bacc.Bacc("TRN2", target_bir_lowering=False, debug=False, num_devices=N_CORES)

    o_d = nc.dram_tensor("o", [BL, O_T, ODIM], BF16, kind="ExternalInput").ap()
    rgb_d = nc.dram_tensor("rgb", [BL, T, 2048], BF16, kind="ExternalInput").ap()
    aud_d = nc.dram_tensor("audio", [BL, T, 128], BF16, kind="ExternalInput").ap()
    gm_d = nc.dram_tensor("group_mask", [B], U8, kind="ExternalInput").ap()

    wd = {}
    for m, dm in (("rgb", 2048), ("audio", 128)):
        for p in "qkv":
            wd[f"{m}_W{p}"] = nc.dram_tensor(f"{m}_W{p}", [dm, DIM], BF16, kind="ExternalInput").ap()
            wd[f"{m}_b{p}"] = nc.dram_tensor(f"{m}_b{p}", [DIM], F32, kind="ExternalInput").ap()
        wd[f"{m}_Wo"] = nc.dram_tensor(f"{m}_Wo", [DIM, DIM], BF16, kind="ExternalInput").ap()
        wd[f"{m}_bo"] = nc.dram_tensor(f"{m}_bo", [DIM], F32, kind="ExternalInput").ap()
        wd[f"{m}_W2"] = nc.dram_tensor(f"{m}_W2", [DIM, DIM], BF16, kind="ExternalInput").ap()
        wd[f"{m}_b2"] = nc.dram_tensor(f"{m}_b2", [DIM], F32, kind="ExternalInput").ap()
    wd["expand_W"] = nc.dram_tensor("expand_W", [DIM, 2 * DIM], BF16, kind="ExternalInput").ap()
    wd["expand_b"] = nc.dram_tensor("expand_b", [2 * DIM], F32, kind="ExternalInput").ap()

    out_d = nc.dram_tensor("out", [1, 6], F32, kind="ExternalOutput").ap()

    import os
    stage = os.environ.get("KSTAGE", "full")
    dbg_d = None
    if stage != "full":
        dbg_d = nc.dram_tensor("dbg", [B, 4 * DIM], F32, kind="ExternalOutput").ap()

    with tile.TileContext(nc) as tc:
        _emit(nc, tc, o_d, rgb_d, aud_d, gm_d, wd, out_d, stage, dbg_d)

    nc.compile()
    return nc


def _emit(nc, tc, o_d, rgb_d, aud_d, gm_d, wd, out_d, stage="full", dbg_d=None):
    stk = ExitStack()
    with stk:
        const = stk.enter_context(tc.tile_pool(name="const", bufs=1))
        persist = stk.enter_context(tc.tile_pool(name="persist", bufs=1))
        ps = stk.enter_context(tc.tile_pool(name="psum", bufs=7, space="PSUM"))
        dram = stk.enter_context(tc.tile_pool(name="dram", bufs=1, space="DRAM"))

        def pst(shape, tag="ps", bufs=None):
            return ps.tile(shape, F32, tag=tag, bufs=bufs, name=tag)

        # ---------------- constants ----------------
        ident = const.tile([128, 128], F32, tag="ident")
        make_identity(nc, ident)
        ones_col_f32 = const.tile([128, 1], F32, tag="ones_col_f32")
        nc.vector.memset(ones_col_f32[:], 1.0)
        ones64_s = const.tile([128, 128], F32, tag="ones64_s")
        nc.vector.memset(ones64_s[:], 0.0)
        nc.vector.memset(ones64_s[0:64, 0:64], 1.0)
        nc.vector.memset(ones64_s[64:128, 64:128], 1.0)
        ones_row_f32 = const.tile([1, 128], F32, tag="ones_row_f32")
        nc.vector.memset(ones_row_f32[:], 1.0)
        ones128 = const.tile([128, 128], F32, tag="ones128")
        nc.vector.memset(ones128[:], 1.0)
        ones_row_bf = const.tile([1, 128], BF16, tag="ones_row_bf")
        nc.vector.tensor_copy(ones_row_bf[:], ones_row_f32[:])
        sel16_s = const.tile([128, BL, BL], F32, tag="sel16_s")
        nc.vector.memset(sel16_s[:], 0.0)
        for b in range(BL):
            nc.vector.memset(sel16_s[:, b, b : b + 1], 1.0)
        sel16 = const.tile([128, BL, BL], BF16, tag="sel16")
        nc.vector.tensor_copy(sel16[:], sel16_s[:])
        ones64_bf = const.tile([64, 64], BF16, tag="ones64_bf")
        nc.vector.tensor_copy(ones64_bf[:], ones64_s[0:64, 0:64])

        g_row_u8 = const.tile([1, B], U8, tag="g_row_u8")
        nc.sync.dma_start(g_row_u8[:], gm_d[None, :])
        g_row = const.tile([1, B], F32, tag="g_row")
        nc.vector.tensor_copy(g_row[:], g_row_u8[:])
        g_col_u8 = const.tile([B, 1], U8, tag="g_col_u8")
        nc.sync.dma_start(g_col_u8[:], gm_d[:, None])
        g_col = const.tile([B, 1], F32, tag="g_col")
        nc.vector.tensor_copy(g_col[:], g_col_u8[:])

        feat_sb = persist.tile([BL, 2 * DIM], F32, tag="feat")
        oo_sb = persist.tile([BL, 2 * DIM], F32, tag="oo")

        # o tiles pool opened early so its space never WAR-blocks on encoder pools
        o_pool = stk.enter_context(tc.tile_pool(name="o_pool", bufs=4))

        # ---------------- rgb encoder ------------------------------------------------
        rgb_enc = _encoder_qkv(nc, tc, pst, persist, const, "rgb", 2048, rgb_d, wd,
                               ident, ones_row_bf)
        _attention(nc, tc, pst, const, "rgb", wd, feat_sb, 0, ones_row_bf,
                   ones64_bf, rgb_enc)

        # expand weights loaded early (small; unblocks the expand chain)
        expw_pool = stk.enter_context(tc.tile_pool(name="expw", bufs=1))
        expw = expw_pool.tile([128, 4, 2 * DIM], BF16, tag="expw")
        nc.gpsimd.dma_start(expw[:], wd["expand_W"].rearrange("(c p) d -> p c d", p=128))
        expb = expw_pool.tile([1, 2 * DIM], BF16, tag="expb")
        nc.gpsimd.dma_start(expb[:], wd["expand_b"][None, :])

        # ---------------- audio encoder qkv (attention deferred past o-mean) ---------
        aud_enc = _encoder_qkv(nc, tc, pst, persist, const, "audio", 128, aud_d, wd,
                               ident, ones_row_bf)

        # ---------------- o-mean (bf16 stream; 2-sample DMAs split Pool/SP) ----------
        om_ps = pst([BL, ODIM], tag="ps_om", bufs=1)
        o_view = o_d.rearrange("b (n p) d -> p b n d", p=128)  # [128, 16, 8, 512]
        PAIR = 2
        for c in range(BL // PAIR):
            o_sb = o_pool.tile([128, PAIR, O_T // 128, ODIM], BF16, tag="o_tile")
            eng = nc.gpsimd if c % 2 == 0 else nc.sync
            eng.dma_start(o_sb[:], o_view[:, PAIR * c : PAIR * (c + 1)])
            for bb in range(PAIR):
                b = PAIR * c + bb
                for j in range(O_T // 128):
                    nc.tensor.matmul(
                        om_ps[:],
                        sel16[:, b, :],
                        o_sb[:, bb, j, :],
                        start=(b == 0 and j == 0),
                        stop=(b == BL - 1 and j == O_T // 128 - 1),
                    )

        om_sb = persist.tile([BL, ODIM], F32, tag="om")
        nc.scalar.activation(om_sb[:], om_ps[:], AF.Copy, scale=1.0 / O_T)
        omT = persist.tile([128, 4, BL], BF16, tag="omT")
        for c in range(4):
            tp = pst([128, BL])
            nc.tensor.transpose(tp[:], om_sb[:, 128 * c : 128 * (c + 1)], ident[:BL, :BL])
            nc.scalar.copy(omT[:, c, :], tp[:])

        # ---------------- expand + normalize -> oo ----------------
        if True:
            oo_ps = []
            for half in range(2):
                pp = pst([BL, DIM])
                for c in range(4):
                    nc.tensor.matmul(pp[:], omT[:, c, :], expw[:, c, 512 * half : 512 * (half + 1)],
                                     start=(c == 0), stop=False)
                nc.tensor.matmul(pp[:], ones_row_bf[:, :BL], expb[:, 512 * half : 512 * (half + 1)],
                                 start=False, stop=True)
                oo_ps.append(pp)
            sq_junk = persist.tile([BL, DIM], F32, tag="sq_junk")
            ss = [persist.tile([BL, 1], F32, tag=f"ss{i}", name=f"ss{i}") for i in range(2)]
            for half in range(2):
                nc.scalar.activation(sq_junk[:], oo_ps[half][:], AF.Square, accum_out=ss[half][:])
            nrm = persist.tile([BL, 1], F32, tag="nrm")
            nc.vector.tensor_tensor(nrm[:], ss[0][:], ss[1][:], AluOpType.add)
            nc.scalar.sqrt(nrm[:], nrm[:])
            nc.vector.tensor_scalar_max(nrm[:], nrm[:], 1e-12)
            rnrm = persist.tile([BL, 1], F32, tag="rnrm")
            nc.vector.reciprocal(rnrm[:], nrm[:])
            for half in range(2):
                nc.vector.tensor_scalar_mul(oo_sb[:, 512 * half : 512 * (half + 1)],
                                            oo_ps[half][:], rnrm[:])

        # ---------------- audio attention (overlaps o-mean/expand tail) -------------
        _attention(nc, tc, pst, const, "audio", wd, feat_sb, DIM, ones_row_bf,
                   ones64_bf, aud_enc)

        if stage == "enc":
            nc.sync.dma_start(dbg_d[0:BL, 0 : 2 * DIM], feat_sb[:])
            return

        if stage == "oenc":
            nc.sync.dma_start(dbg_d[0:BL, 0 : 2 * DIM], feat_sb[:])
            nc.sync.dma_start(dbg_d[0:BL, 2 * DIM :], oo_sb[:])
            return

        # ---------------- AllGather ----------------
        ag_in = dram.tile([BL, 4 * DIM], F32)
        ag_out = dram.tile([B, 4 * DIM], F32)
        nc.sync.dma_start(ag_in[:, : 2 * DIM], feat_sb[:])
        nc.sync.dma_start(ag_in[:, 2 * DIM :], oo_sb[:])
        import os
        if os.environ.get("KTIME"):
            # collective-free stand-in for TimelineSim (cost model can't model
            # collectives); timing-equivalent except the ~15us AllGather.
            nc.sync.dma_start(ag_out[0:BL, :], ag_in[:])
        else:
            nc.gpsimd.collective_compute(
                "AllGather",
                AluOpType.bypass,
                replica_groups=[list(range(N_CORES))],
                ins=[ag_in.opt()],
                outs=[ag_out.opt()],
            )

        # ---------------- ranking ----------------
        with tc.tile_pool(name="rank", bufs=1) as rank_pool:
            emb = rank_pool.tile([B, 4 * DIM], F32, tag="emb")
            nc.sync.dma_start(emb[:], ag_out[:])

            if stage == "ag":
                nc.sync.dma_start(dbg_d[:], emb[:])
                return

            # transpose emb -> embT [128, 16, 128]; chunks 0..7 featT, 8..15 ooT
            embT = rank_pool.tile([128, 16, 128], F32, tag="embT")
            for grp4 in range(4):
                tp = pst([128, 512])
                for j in range(4):
                    c = 4 * grp4 + j
                    nc.tensor.transpose(tp[:, 128 * j : 128 * (j + 1)],
                                        emb[:, 128 * c : 128 * (c + 1)], ident[:])
                nc.scalar.copy(embT[:, 4 * grp4 : 4 * grp4 + 4, :],
                               tp[:].rearrange("p (j c) -> p j c", j=4))

            G_ps = pst([B, B])
            for c in range(8):
                nc.tensor.matmul(G_ps[:], embT[:, 8 + c, :], embT[:, c, :],
                                 start=(c == 0), stop=(c == 7))
            G_sb = rank_pool.tile([B, B], F32, tag="G_sb")
            nc.scalar.copy(G_sb[:], G_ps[:])

            if stage == "rank1":
                nc.sync.dma_start(dbg_d[:, 0:B], G_sb[:])
                return

            junk = rank_pool.tile([B, B], F32, tag="junk")
            diag = rank_pool.tile([B, 1], F32, tag="diag")
            nc.vector.tensor_tensor(junk[:], G_sb[:], ident[:], AluOpType.mult)
            nc.vector.reduce_sum(diag[:], junk[:], axis=AX.X)
            mdiag = rank_pool.tile([B, 1], F32, tag="mdiag")
            nc.vector.tensor_scalar(mdiag[:], diag[:], -1.0, MARGIN,
                                    AluOpType.mult, AluOpType.add)

            Gt_ps = pst([B, B])
            nc.tensor.transpose(Gt_ps[:], G_sb[:], ident[:])
            Gt_sb = rank_pool.tile([B, B], F32, tag="Gt_sb")
            nc.scalar.copy(Gt_sb[:], Gt_ps[:])

            if stage == "rank1b":
                nc.sync.dma_start(dbg_d[:, 0:B], Gt_sb[:])
                nc.sync.dma_start(dbg_d[:, B : B + 1], diag[:])
                return

            # broadcast g along partitions: gb[m, n] = g[n], via colsums of a
            # zero-padded one-row matrix (K=1 matmuls are avoided).
            g_pad = rank_pool.tile([B, B], F32, tag="g_pad")
            nc.vector.memset(g_pad[:], 0.0)
            nc.vector.tensor_copy(g_pad[0:1, :], g_row[:])
            gb_ps = pst([B, B])
            nc.tensor.matmul(gb_ps[:], ones128[:], g_pad[:], start=True, stop=True)
            gneg_sb = rank_pool.tile([B, B], F32, tag="gneg_sb")
            nc.vector.tensor_scalar(gneg_sb[:], gb_ps[:], 1e30, -1e30,
                                    AluOpType.mult, AluOpType.add)

            stack = rank_pool.tile([B, 6], F32, tag="stack")
            Gm = rank_pool.tile([B, B], F32, tag="Gm")
            rmax = rank_pool.tile([B, 1], F32, tag="rmax")
            top = rank_pool.tile([B, 1], F32, tag="top")
            w = rank_pool.tile([B, 1], F32, tag="w")
            sel = rank_pool.tile([B, 1], F32, tag="sel")
            eq = rank_pool.tile([B, 1], F32, tag="eq")
            colv = rank_pool.tile([B, 1], F32, tag="colv")

            for di, (Gsrc, GsrcT) in enumerate(((G_sb, Gt_sb), (Gt_sb, G_sb))):
                T_sb = rank_pool.tile([B, B], F32, tag=f"T{di}")
                nc.scalar.activation(T_sb[:], Gsrc[:], AF.Relu, bias=mdiag[:])
                nc.vector.tensor_tensor(junk[:], T_sb[:], gb_ps[:], AluOpType.mult)
                nc.vector.reduce_sum(w[:], junk[:], axis=AX.X)
                nc.vector.tensor_tensor(stack[:, di : di + 1], w[:], g_col[:], AluOpType.mult)
                nc.vector.tensor_tensor(Gm[:], Gsrc[:], gneg_sb[:], AluOpType.add)
                nc.vector.reduce_max(rmax[:], Gm[:], axis=AX.X)
                nc.vector.tensor_tensor(top[:], diag[:], rmax[:], AluOpType.is_ge)
                # sel[i] = sum_b Gsrc[i,b]*g[b] as an N=1 matmul off GsrcT
                sel_ps = pst([B, 1])
                nc.tensor.matmul(sel_ps[:], GsrcT[:], g_col[:], start=True, stop=True)
                nc.vector.tensor_tensor(sel[:], sel_ps[:], g_col[:], AluOpType.mult)
                nc.vector.tensor_scalar(eq[:], sel[:], 0.0, None, AluOpType.is_equal)
                nc.vector.tensor_scalar(colv[:], eq[:], -1.0, 1.0,
                                        AluOpType.mult, AluOpType.add)
                nc.vector.tensor_copy(stack[:, 4 + di : 5 + di], colv[:])
                nc.vector.tensor_tensor(stack[:, 2 + di : 3 + di], colv[:], top[:],
                                        AluOpType.mult)

            if stage == "rank2":
                nc.sync.dma_start(dbg_d[:, 0:6], stack[:])
                nc.sync.dma_start(dbg_d[:, 8:136], Gt_sb[:])
                return

            # column-sums of the 6 stacked partials; the final scalar math
            # (divides by g-dependent counts) runs on host during unshard.
            S_ps = pst([1, 6])
            nc.tensor.matmul(S_ps[:], ones_col_f32[:], stack[:], start=True, stop=True)
            S_sb = rank_pool.tile([1, 6], F32, tag="S_sb")
            nc.vector.tensor_copy(S_sb[:], S_ps[:])

            if stage == "rank3":
                nc.sync.dma_start(dbg_d[0:1, 0:6], S_sb[:])
                return

            nc.sync.dma_start(out_d[:], S_sb[:])


def _encoder_qkv(nc, tc, pst, persist, const, mod, dm, x_d, wd,
                 ident, ones_row_bf):
    """Self-attention encoder, projection part: computes qT/kT/v_sb."""
    K = dm // 128
    n_tt = TOK // 128  # 8

    enc_pool_cm = tc.tile_pool(name=f"enc_{mod}", bufs=1)
    enc = enc_pool_cm.__enter__()
    qT = enc.tile([128, HEADS, TOK], BF16, tag="qT")
    kT = enc.tile([128, HEADS, TOK], BF16, tag="kT")
    v_sb = enc.tile([128, n_tt, DIM], BF16, tag="v_sb")
    poolT = enc.tile([128, HEADS, BL], BF16, tag="poolT")

    with ExitStack() as estk:
        xT_pool = estk.enter_context(tc.tile_pool(name=f"xT_{mod}", bufs=1))
        xT = xT_pool.tile([128, K, TOK], BF16, tag="xT")
        flat = x_d.rearrange("b t d -> (b t) d")
        # xT[d, tok] straight from DRAM via HWDGE xbar-transpose (bf16).
        # One transpose-DMA per 128-token row-block so downstream matmuls can
        # start after the first block lands; alternate SP/ACT queues.
        if K > 1:
            for tt in range(n_tt):
                eng = nc.sync if tt % 2 == 0 else nc.scalar
                eng.dma_start(xT[:, :, 128 * tt : 128 * (tt + 1)],
                              flat[128 * tt : 128 * (tt + 1), :], transpose=True)
        else:
            nc.sync.dma_start(xT[:, 0, :], flat[:, :], transpose=True)

        # v: lhsT = xT token-tile (stationary), rhs = Wv k-rows (moving)
        with tc.tile_pool(name=f"wv_{mod}", bufs=1) as wv_pool:
            wv = wv_pool.tile([128, K, DIM], BF16, tag="wv")
            wv_view = wd[f"{mod}_Wv"].rearrange("(kc p) d -> p kc d", p=128)
            # chunked so the first v matmuls unblock after ~1/4 of the load
            kchunk = max(K // 4, 1)
            for c0 in range(0, K, kchunk):
                nc.gpsimd.dma_start(wv[:, c0 : c0 + kchunk, :],
                                    wv_view[:, c0 : c0 + kchunk, :])
            bv = wv_pool.tile([1, DIM], BF16, tag="bv")
            nc.gpsimd.dma_start(bv[:], wd[f"{mod}_bv"][None, :])
            for tt in range(n_tt):
                pv = pst([128, DIM])
                for kc in range(K):
                    nc.tensor.matmul(pv[:], xT[:, kc, 128 * tt : 128 * (tt + 1)], wv[:, kc, :],
                                     start=(kc == 0), stop=False)
                nc.tensor.matmul(pv[:], ones_row_bf[:], bv[:], start=False, stop=True)
                nc.vector.tensor_copy(v_sb[:, tt, :], pv[:])

        # q, k: lhsT = W column-block (stationary), rhs = xT (moving) -> [d, tok]
        bq_sb = const.tile([128, HEADS], F32, tag=f"bq_{mod}")
        nc.sync.dma_start(bq_sb[:], wd[f"{mod}_bq"].rearrange("(o p) -> p o", p=128))
        bk_sb = const.tile([128, HEADS], F32, tag=f"bk_{mod}")
        nc.sync.dma_start(bk_sb[:], wd[f"{mod}_bk"].rearrange("(o p) -> p o", p=128))
        with tc.tile_pool(name=f"wcol_{mod}", bufs=2) as wcol_pool:
            for pname, outT, b_sb in (("q", qT, bq_sb), ("k", kT, bk_sb)):
                w_d = wd[f"{mod}_W{pname}"].rearrange("(kc p) d -> p kc d", p=128)
                if K == 1:
                    wfull = wcol_pool.tile([128, DIM], BF16, tag="wfull", name="wfull")
                    nc.gpsimd.dma_start(wfull[:], w_d[:, 0, :])
                for dt_ in range(HEADS):
                    if K == 1:
                        wcol = wfull[:, None, 128 * dt_ : 128 * (dt_ + 1)]
                    else:
                        wcol = wcol_pool.tile([128, K, 128], BF16, tag="wcol",
                                              name="wcol")
                        nc.gpsimd.dma_start(wcol[:],
                                            w_d[:, :, 128 * dt_ : 128 * (dt_ + 1)])
                    for blk in range(TOK // 512):
                        pq = pst([128, 512])
                        for kc in range(K):
                            nc.tensor.matmul(pq[:], wcol[:, kc, :],
                                             xT[:, kc, 512 * blk : 512 * (blk + 1)],
                                             start=(kc == 0), stop=(kc == K - 1))
                        nc.scalar.activation(outT[:, dt_, 512 * blk : 512 * (blk + 1)], pq[:],
                                             AF.Identity, bias=b_sb[:, dt_ : dt_ + 1])

    return {"cm": enc_pool_cm, "qT": qT, "kT": kT, "v_sb": v_sb, "poolT": poolT}


def _attention(nc, tc, pst, const, mod, wd, feat_sb, feat_off, ones_row_bf,
               ones64_bf, enc):
    """Attention + pooling + out-proj; software-pipelined emission so the
    per-(grp,head) exp->sum->recip->mult chain overlaps across iterations.
    Writes feat_sb[:, feat_off:feat_off+512]."""
    qT, kT, v_sb, poolT = enc["qT"], enc["kT"], enc["v_sb"], enc["poolT"]
    scale = 1.0 / math.sqrt(HD)
    with ExitStack() as lstk:
        late = lstk.enter_context(tc.tile_pool(name=f"late_{mod}", bufs=1))
        avT = late.tile([128, HEADS, TOK], BF16, tag="avT")
        wo_pool = lstk.enter_context(tc.tile_pool(name=f"wo_{mod}", bufs=1))
        wo = wo_pool.tile([128, HEADS, DIM], BF16, tag="wo")
        nc.gpsimd.dma_start(wo[:], wd[f"{mod}_Wo"].rearrange("(h p) d -> p h d", p=128))
        w2 = wo_pool.tile([128, HEADS, DIM], BF16, tag="w2")
        nc.gpsimd.dma_start(w2[:], wd[f"{mod}_W2"].rearrange("(c p) d -> p c d", p=128))
        b2 = wo_pool.tile([1, DIM], BF16, tag="b2")
        nc.gpsimd.dma_start(b2[:], wd[f"{mod}_b2"][None, :])
        bo_sb = const.tile([128, HEADS], F32, tag=f"bo_{mod}")
        nc.sync.dma_start(bo_sb[:], wd[f"{mod}_bo"].rearrange("(o p) -> p o", p=128))
        ap = lstk.enter_context(tc.tile_pool(name=f"attn_{mod}", bufs=3))

        NIT = (BL // 8) * HEADS
        v8s, exps_t, sT8_t, rs_t, aT8_t = {}, {}, {}, {}, {}

        def stage_a(t):  # scores + exp
            grp, h = divmod(t, HEADS)
            if h == 0:
                v8 = ap.tile([64, 8, DIM], BF16, tag="v8")
                v8v = v8[:].rearrange("p (ul half) d -> p ul half d", half=2)
                nc.sync.dma_start(v8v[:, :, 0, :], v_sb[0:64, 4 * grp : 4 * grp + 4, :])
                nc.sync.dma_start(v8v[:, :, 1, :], v_sb[64:128, 4 * grp : 4 * grp + 4, :])
                v8s[grp] = v8
            sT8 = pst([64, 512])
            for i in range(8):
                b = 8 * grp + i
                nc.tensor.matmul(sT8[:, 64 * i : 64 * (i + 1)],
                                 kT[:, h, 64 * b : 64 * (b + 1)],
                                 qT[:, h, 64 * b : 64 * (b + 1)],
                                 start=True, stop=True)
            exps = ap.tile([64, 512], BF16, tag="exps")
            nc.scalar.activation(exps[:], sT8[:], AF.Exp, scale=scale)
            exps_t[t] = exps

        def stage_b(t):  # rowsum + reciprocal + normalize
            rs = pst([64, 512])
            nc.tensor.matmul(rs[:], ones64_bf[:], exps_t[t][:], start=True, stop=True)
            rrs = ap.tile([64, 512], F32, tag="rrs")
            nc.vector.reciprocal(rrs[:], rs[:])
            aT8 = ap.tile([64, 512], BF16, tag="aT8")
            nc.vector.tensor_tensor(aT8[:], exps_t[t][:], rrs[:], AluOpType.mult)
            aT8_t[t] = aT8

        def stage_c(t):  # attention-weighted v + copy out
            grp, h = divmod(t, HEADS)
            v8 = v8s[grp]
            avp = pst([128, 512])
            for i in range(8):
                nc.tensor.matmul(avp[:, 64 * i : 64 * (i + 1)],
                                 v8[:, i, 128 * h : 128 * (h + 1)],
                                 aT8_t[t][:, 64 * i : 64 * (i + 1)],
                                 start=True, stop=True)
            nc.vector.tensor_copy(avT[:, h, 512 * grp : 512 * (grp + 1)], avp[:])

        for t in range(NIT + 2):
            if t < NIT:
                stage_a(t)
            if 0 <= t - 1 < NIT:
                stage_b(t - 1)
            if 0 <= t - 2 < NIT:
                stage_c(t - 2)

        # out-proj (transposed) + time pooling + W2
        _proj_w2(nc, tc, pst, wo_pool, mod, feat_sb, feat_off, avT, poolT,
                 ones_row_bf, wo, w2, b2, bo_sb)

    enc["cm"].__exit__(None, None, None)


def _proj_w2(nc, tc, pst, wo_pool, mod, feat_sb, feat_off, avT, poolT,
             ones_row_bf, wo, w2, b2, bo_sb):
    if True:
        NB = TOK // 512
        its = [(dt_, blk) for dt_ in range(HEADS) for blk in range(NB)]
        pps = {}

        def proj_mm(i):
            dt_, blk = its[i]
            pp = pst([128, 512])
            for h in range(HEADS):
                nc.tensor.matmul(pp[:], wo[:, h, 128 * dt_ : 128 * (dt_ + 1)],
                                 avT[:, h, 512 * blk : 512 * (blk + 1)],
                                 start=(h == 0), stop=(h == HEADS - 1))
            pps[i] = pp

        def proj_red(i):
            dt_, blk = its[i]
            red = wo_pool.tile([128, 8], F32, tag="red", name=f"red{i}")
            nc.vector.reduce_sum(red[:], pps[i][:].rearrange("p (s t) -> p s t", t=T),
                                 axis=AX.X)
            nc.vector.tensor_scalar(poolT[:, dt_, 8 * blk : 8 * blk + 8], red[:],
                                    1.0 / T, bo_sb[:, dt_ : dt_ + 1],
                                    AluOpType.mult, AluOpType.add)

        for i in range(len(its) + 1):
            if i < len(its):
                proj_mm(i)
            if i >= 1:
                proj_red(i - 1)

        pf = pst([BL, DIM])
        for c in range(HEADS):
            nc.tensor.matmul(pf[:], poolT[:, c, :], w2[:, c, :], start=(c == 0), stop=False)
        nc.tensor.matmul(pf[:], ones_row_bf[:, :BL], b2[:], start=False, stop=True)
        nc.scalar.copy(feat_sb[:, feat_off : feat_off + DIM], pf[:])


def kernel(**inputs):
    if "runner" not in _CACHE:
        _CACHE["runner"] = _make_runner()
    return _CACHE["runner"](inputs)


def _make_runner():
    nc = _build()
    import jax
    from jax.sharding import Mesh, PartitionSpec
    from jax.experimental.shard_map import shard_map
    from concourse import bass2jax

    bass2jax.install_neuronx_cc_hook()

    partition_name = nc.partition_id_tensor.name if nc.partition_id_tensor else None
    in_names, out_names, out_avals, zero_outs = [], [], [], []
    in_dtypes = {}
    for alloc in nc.m.functions[0].allocations:
        if not isinstance(alloc, mybir.MemoryLocationSet):
            continue
        name = alloc.memorylocations[0].name
        if alloc.kind == "ExternalInput":
            if name != partition_name:
                in_names.append(name)
                in_dtypes[name] = mybir.dt.np(alloc.dtype)
        elif alloc.kind == "ExternalOutput":
            out_names.append(name)
            shape = tuple(alloc.tensor_shape)
            dtype = mybir.dt.np(alloc.dtype)
            out_avals.append(jax.core.ShapedArray(shape, dtype))
            zero_outs.append(np.zeros(shape, dtype))
    n_params = len(in_names)
    all_in_names = list(in_names) + list(out_names)
    if partition_name is not None:
        all_in_names.append(partition_name)

    def _body(*args):
        operands = list(args)
        if partition_name is not None:
            operands.append(bass2jax.partition_id_tensor())
        outs = bass2jax._bass_exec_p.bind(
            *operands,
            out_avals=tuple(out_avals),
            in_names=tuple(all_in_names),
            out_names=tuple(out_names),
            lowering_input_output_aliases=(),
            sim_require_finite=True,
            sim_require_nnan=True,
            nc=nc,
        )
        return tuple(outs)

    devices = jax.devices()[:N_CORES]
    mesh = Mesh(np.asarray(devices), ("core",))
    in_specs = (PartitionSpec("core"),) * (n_params + len(out_names))
    out_specs = (PartitionSpec("core"),) * len(out_names)
    sharded = jax.jit(
        shard_map(_body, mesh=mesh, in_specs=in_specs, out_specs=out_specs,
                  check_rep=False),
        keep_unused=True,
    )

    out_idx = out_names.index("out")

    def run(inputs):
        per_core = _shard_inputs(inputs, in_dtypes)
        concat_in = [
            np.concatenate([per_core[c][name] for c in range(N_CORES)], axis=0)
            for name in in_names
        ]
        concat_zeros = [
            np.zeros((N_CORES * z.shape[0], *z.shape[1:]), z.dtype) for z in zero_outs
        ]
        out_arrs = sharded(*concat_in, *concat_zeros)
        run.last_outputs = {n: np.asarray(out_arrs[i]) for i, n in enumerate(out_names)}
        out = run.last_outputs["out"]  # [8, 6] column-sum partials, identical rows
        S = np.asarray(out[0], np.float64)
        sg = float(np.asarray(inputs["group_mask"]).astype(bool).sum())
        num = S[0] + S[1] - 2.0 * MARGIN * sg
        d1 = max(sg - 1.0, 1.0)
        d2 = max(min(max(sg - 1.0, 0.0), 1.0) * sg, 1.0)
        loss = num / d1 / d2
        acc = 0.5 * (S[2] / max(S[4], 1.0) + S[3] / max(S[5], 1.0))
        return np.float32(loss), np.float32(acc)

    run.sharded = sharded
    run.in_names = in_names
    run.in_dtypes = in_dtypes
    run.zero_outs = zero_outs
    run.nc = nc
    return run


def _shard_inputs(inputs, in_dtypes=None):
    if in_dtypes is None:
        in_dtypes = _CACHE["runner"].in_dtypes
    per_core = []
    gm = np.ascontiguousarray(np.asarray(inputs["group_mask"]).astype(np.uint8))
    shared = {}
    for k, v in inputs.items():
        if k not in ("o", "rgb", "audio", "group_mask"):
            shared[k] = np.ascontiguousarray(
                np.asarray(v).astype(in_dtypes.get(k, np.float32)))
    o = np.asarray(inputs["o"]).astype(in_dtypes.get("o", np.float32))
    rgb = np.asarray(inputs["rgb"]).astype(in_dtypes.get("rgb", np.float32))
    audio = np.asarray(inputs["audio"]).astype(in_dtypes.get("audio", np.float32))
    for c in range(N_CORES):
        sl = slice(BL * c, BL * (c + 1))
        m = {
            "o": np.ascontiguousarray(o[sl]),
            "rgb": np.ascontiguousarray(rgb[sl]),
            "audio": np.ascontiguousarray(audio[sl]),
            "group_mask": gm,
        }
        m.update(shared)
        per_core.append(m)
    return per_core
